# revision 3
# baseline (speedup 1.0000x reference)
"""Trainium2 Bass kernel for the 4-step shift-only MAF (MADE) chain.

Strategy (v2): tensor-parallel over hidden/feature dims across 8 NeuronCores
(column-parallel every layer), activations transposed [features, batch].
The inter-step `z[:, ::-1]` permute is folded into host-side weight prep.
After each layer an AllGather rebuilds the full activation.

v2 changes vs v1 (62.6us / ~194us steady-state):
- fp8 e4m3 weights AND activations with DoubleRow matmuls (256-deep
  contraction per instruction): 4x fewer weight bytes than f32, 2x fewer
  tensor instructions than bf16.
- Block-triangular MADE-mask tile skipping with SPMD-uniform slot padding:
  each core owns column-tiles (c, 15-c) of every hidden layer, so the
  per-core k-pair slot budget is uniform (L0: 2+4, L1/L2: 4+8, L3: 8 of
  dense 4/8/16/8) -> ~75% of dense weight bytes + matmul instructions.
- Weight DMAs split across two DGE queues (scalar + gpsimd) so streaming
  isn't serialized behind one queue; AG bounce copies stay on sync.
- All step biases loaded once up front.

Per-core device program (SPMD; per-core data via in_maps):
  zT [P,8,B] fp8 (z*SA), zloc [P,B] f32 (z*SA of own block) start as x.
  Per step s: L0/L1/L2 column-parallel with Relu(psum/SW + b*SA) -> fp8
  hloc [P,2,B] -> AG -> gathered hT [P,2,4,2,B] (halfgroup, pairidx,
  member, batch). L3 -> shift; zloc -= shift; AG z (not last step).
  Finally sq = ones.T @ (zloc/SA)^2 -> [1,B]; host sums partials.
"""

import os
import sys

import numpy as np

for _p in ("/opt/trn_rl_repo", "/opt/trn_rl_repo/concourse"):
    if _p not in sys.path:
        sys.path.insert(0, _p)

B = 100
DIM = 1024
H = 2048
STEPS = 4
NC = 8
P = 128
KD = DIM // P   # 8 z k-tiles (4 pairs)
KH = H // P     # 16 h k-tiles (8 pairs)
LOG_2PI = float(np.log(2.0 * np.pi))
F32 = np.float32

SW = 64.0   # weight scale into fp8
SA = 16.0   # activation scale into fp8
FP8_MAX = 240.0  # TRN FP8_EXP4 max normal

# slot budgets (pairs of 128-row k-tiles) per layer: [A-slots, B-slots]
SL0 = (2, 4)
SL1 = (4, 8)
SL3 = 8


def _made_mask(n_in, n_out, exclusive):
    d_in, d_out = n_in // DIM, n_out // DIM
    deg_in = np.arange(n_in) // d_in
    deg_out = np.arange(n_out) // d_out
    if exclusive:
        m = deg_out[None, :] > deg_in[:, None]
    else:
        m = deg_out[None, :] >= deg_in[:, None]
    return m.astype(F32)


def _q8(x, np8):
    return np.clip(x * SW, -FP8_MAX, FP8_MAX).astype(np8)


def _l1_slots(c):
    """(slot -> (ktile_j0, ktile_j1)) for the 12 W1/W2 slots of core c."""
    out = []
    for u in range(4):                      # A: pairs 0..3
        out.append((2 * u, 2 * u + 1))
    for u in range(4):                      # B halfgroup 0: pairs 0..3
        out.append((2 * u, 2 * u + 1))
    for u in range(4):                      # B halfgroup 1: pair 7-u
        out.append((15 - 2 * u, 14 - 2 * u))
    return out


def _l0_slots(parity):
    """slot -> (ktile_j0, ktile_j1) for the 6 W0 slots (parity = s%2)."""
    pa = (0, 1) if parity == 0 else (3, 2)
    out = [(2 * p, 2 * p + 1) for p in pa]          # A: 2 slots
    out += [(2 * u, 2 * u + 1) for u in range(4)]   # B: pairs 0..3
    return out


def _l3_slots():
    out = [(2 * u, 2 * u + 1) for u in range(4)]        # halfgroup 0
    out += [(15 - 2 * u, 14 - 2 * u) for u in range(4)]  # halfgroup 1
    return out


def _prep_inputs(x, W0, b0, W1, b1, W2, b2, W3, b3):
    """Host-side: mask, flip-fold, quantize, shard, pack slot layouts."""
    from concourse import mybir
    np8 = mybir.dt.np(mybir.dt.float8e4)

    M0 = _made_mask(DIM, H, True)
    M1 = _made_mask(H, H, False)
    M3 = _made_mask(H, DIM, False)

    xT = np.ascontiguousarray(x.T.astype(F32))              # [1024, 100]
    xts = np.clip(xT * SA, -FP8_MAX, FP8_MAX)
    xt_arr = np.ascontiguousarray(
        xts.reshape(KD, P, B).transpose(1, 0, 2)).astype(np8)  # [128,8,100]

    W0e, W1e, W2e, W3e, b3e = [], [], [], [], []
    for s in range(STEPS):
        w0 = W0[s] * M0
        if s % 2 == 1:
            w0 = w0[::-1, :]
        w3 = W3[s] * M3
        b3s = b3[s]
        if s % 2 == 1:
            w3 = w3[:, ::-1]
            b3s = b3s[::-1]
        W0e.append(np.ascontiguousarray(w0))
        W1e.append(W1[s] * M1)
        W2e.append(W2[s] * M1)
        W3e.append(np.ascontiguousarray(w3))
        b3e.append(np.ascontiguousarray(b3s))

    l1slots = _l1_slots(0)
    l3slots = _l3_slots()

    in_maps = []
    for c in range(NC):
        ca, cb = c, 15 - c                      # col-tile indices
        sa_ = slice(P * ca, P * (ca + 1))
        sb_ = slice(P * cb, P * (cb + 1))

        # W0: [S, P, 6, 2, P]
        w0c = np.zeros((STEPS, P, sum(SL0), 2, P), dtype=np8)
        for s in range(STEPS):
            slots = _l0_slots(s % 2)
            for i, (k0, k1) in enumerate(slots):
                cols = sa_ if i < SL0[0] else sb_
                w0c[s, :, i, 0, :] = _q8(W0e[s][P * k0:P * (k0 + 1), cols], np8)
                w0c[s, :, i, 1, :] = _q8(W0e[s][P * k1:P * (k1 + 1), cols], np8)

        def pack_h(We):
            wc = np.zeros((STEPS, P, sum(SL1), 2, P), dtype=np8)
            for s in range(STEPS):
                for i, (k0, k1) in enumerate(l1slots):
                    cols = sa_ if i < SL1[0] else sb_
                    wc[s, :, i, 0, :] = _q8(We[s][P * k0:P * (k0 + 1), cols], np8)
                    wc[s, :, i, 1, :] = _q8(We[s][P * k1:P * (k1 + 1), cols], np8)
            return wc

        w1c = pack_h(W1e)
        w2c = pack_h(W2e)

        # W3: [S, P, 8, 2, P]; m-tile = z-block c
        zc = slice(P * c, P * (c + 1))
        w3c = np.zeros((STEPS, P, SL3, 2, P), dtype=np8)
        for s in range(STEPS):
            for i, (k0, k1) in enumerate(l3slots):
                w3c[s, :, i, 0, :] = _q8(W3e[s][P * k0:P * (k0 + 1), zc], np8)
                w3c[s, :, i, 1, :] = _q8(W3e[s][P * k1:P * (k1 + 1), zc], np8)

        # biases: [S, P, 7] f32, cols = b0A,b0B,b1A,b1B,b2A,b2B,b3 (all *SA)
        bc = np.zeros((STEPS, P, 7), dtype=F32)
        for s in range(STEPS):
            bc[s, :, 0] = b0[s][sa_] * SA
            bc[s, :, 1] = b0[s][sb_] * SA
            bc[s, :, 2] = b1[s][sa_] * SA
            bc[s, :, 3] = b1[s][sb_] * SA
            bc[s, :, 4] = b2[s][sa_] * SA
            bc[s, :, 5] = b2[s][sb_] * SA
            bc[s, :, 6] = b3e[s][zc] * SA

        wall = np.concatenate([w0c, w1c, w2c, w3c], axis=2)  # [S,P,38,2,P]
        in_maps.append({
            "xt": np.ascontiguousarray(xt_arr),
            "xloc": np.ascontiguousarray(xT[zc, :] * SA),
            "w": np.ascontiguousarray(wall),
            "bias": np.ascontiguousarray(bc),
        })
    return in_maps




# ---------------- DP8 (pure data-parallel) variant ----------------
BL = 16           # padded per-core batch (8 * 16 = 128 >= 100)


# ---- dp8v2: tight mixed pair/single slot packing ----
def _v2_layer_slots(n_ktiles_of_v, ncols, from_top=False, nk=None):
    """Per col-tile v: list of ('p', k0) DoubleRow pairs / ('s', k) singles."""
    out = []
    for v in range(ncols):
        n = n_ktiles_of_v(v)
        slots = []
        if not from_top:
            k = 0
            while n - k >= 2:
                slots.append(("p", k))
                k += 2
            if k < n:
                slots.append(("s", k))
        else:
            hi = nk
            lo = nk - n
            k = hi
            while k - lo >= 2:
                slots.append(("p", k - 2))
                k -= 2
            if k > lo:
                slots.append(("s", lo))
        out.append((v, slots))
    return out


def _dp8v2_slots(parity):
    """[(layer, v, slots)] for one step. layer in 0..3."""
    table = []
    # L0: col v needs n = v//2+1 ktiles; parity 1 flip-folded -> top tiles
    l0 = _v2_layer_slots(lambda v: v // 2 + 1, 16, from_top=(parity == 1), nk=8)
    for v, sl in l0:
        table.append((0, v, sl))
    # L1/L2: col v needs ktiles 0..v
    l1 = _v2_layer_slots(lambda v: v + 1, 16)
    for ly in (1, 2):
        for v, sl in l1:
            table.append((ly, v, sl))
    # L3: parity0 col v needs ktiles 0..2v+1; parity1 col v = orig 7-v
    def l3n(v):
        vv = v if parity == 0 else v  # cols already flip-folded host-side
        return 2 * vv + 2 if parity == 0 else 2 * (7 - vv) + 2
    l3 = _v2_layer_slots(l3n, 8)
    for v, sl in l3:
        table.append((3, v, sl))
    return table


def _dp8v2_counts(parity):
    np_, ns_ = 0, 0
    for _, _, sl in _dp8v2_slots(parity):
        for kind, _ in sl:
            if kind == "p":
                np_ += 1
            else:
                ns_ += 1
    return np_, ns_


V2_NP, V2_NS = _dp8v2_counts(0)     # pairs, singles per step (parity-invt)
assert (V2_NP, V2_NS) == _dp8v2_counts(1), (_dp8v2_counts(0), _dp8v2_counts(1))


def _prep_inputs_dp8v2(x, W0, b0, W1, b1, W2, b2, W3, b3):
    from concourse import mybir
    np8 = mybir.dt.np(mybir.dt.float8e4)

    M0 = _made_mask(DIM, H, True)
    M1 = _made_mask(H, H, False)
    M3 = _made_mask(H, DIM, False)

    W0e, W1e, W2e, W3e, b3e = [], [], [], [], []
    for s in range(STEPS):
        w0 = W0[s] * M0
        if s % 2 == 1:
            w0 = w0[::-1, :]
        w3 = W3[s] * M3
        b3s = b3[s]
        if s % 2 == 1:
            w3 = w3[:, ::-1]
            b3s = b3s[::-1]
        W0e.append(np.ascontiguousarray(w0))
        W1e.append(W1[s] * M1)
        W2e.append(W2[s] * M1)
        W3e.append(np.ascontiguousarray(w3))
        b3e.append(np.ascontiguousarray(b3s))

    wp = np.zeros((STEPS, P, V2_NP, 2, P), dtype=np8)
    ws = np.zeros((STEPS, P, V2_NS, P), dtype=np8)
    bc = np.zeros((STEPS, P, 56), dtype=F32)
    for s in range(STEPS):
        Wl = [W0e[s], W1e[s], W2e[s], W3e[s]]
        ip = 0
        isg = 0
        for ly, v, sl in _dp8v2_slots(s % 2):
            We = Wl[ly]
            cols = slice(P * v, P * (v + 1))
            for kind, k0 in sl:
                if kind == "p":
                    wp[s, :, ip, 0, :] = _q8(We[P * k0:P * (k0 + 1), cols], np8)
                    wp[s, :, ip, 1, :] = _q8(We[P * (k0 + 1):P * (k0 + 2), cols],
                                             np8)
                    ip += 1
                else:
                    ws[s, :, isg, :] = _q8(We[P * k0:P * (k0 + 1), cols], np8)
                    isg += 1
        assert ip == V2_NP and isg == V2_NS, (ip, isg)
        for v in range(16):
            bc[s, :, v] = b0[s][P * v:P * (v + 1)] * SA
            bc[s, :, 16 + v] = b1[s][P * v:P * (v + 1)] * SA
            bc[s, :, 32 + v] = b2[s][P * v:P * (v + 1)] * SA
        for v in range(8):
            bc[s, :, 48 + v] = b3e[s][P * v:P * (v + 1)] * SA
    wp = np.ascontiguousarray(wp)
    ws = np.ascontiguousarray(ws)
    bc = np.ascontiguousarray(bc)

    xp = np.zeros((NC * BL, DIM), dtype=F32)
    xp[:B] = x * SA
    in_maps = []
    for c in range(NC):
        xc = xp[BL * c:BL * (c + 1)].T          # [DIM, BL]
        zt = xc.reshape(KD, P, BL).transpose(1, 0, 2)   # [P, KD, BL]
        in_maps.append({
            "xloc": np.ascontiguousarray(zt),
            "xt": np.ascontiguousarray(
                np.clip(zt, -FP8_MAX, FP8_MAX).astype(np8)),
            "wp": wp,
            "ws": ws,
            "bias": bc,
        })
    return in_maps


def _build_module_dp8v2(repeat=1):
    from concourse import bass, bacc, tile, mybir

    f32 = mybir.dt.float32
    fp8 = mybir.dt.float8e4
    Relu = mybir.ActivationFunctionType.Relu
    Square = mybir.ActivationFunctionType.Square
    DR = mybir.MatmulPerfMode.DoubleRow

    nc = bacc.Bacc("TRN2", target_bir_lowering=False, debug=False,
                   num_devices=NC)

    xt_d = nc.dram_tensor("xt", [P, KD, BL], fp8, kind="ExternalInput")
    xloc_d = nc.dram_tensor("xloc", [P, KD, BL], f32, kind="ExternalInput")
    wp_d = nc.dram_tensor("wp", [STEPS, P, V2_NP, 2, P], fp8,
                          kind="ExternalInput")
    ws_d = nc.dram_tensor("ws", [STEPS, P, V2_NS, P], fp8,
                          kind="ExternalInput")
    b_d = nc.dram_tensor("bias", [STEPS, P, 56], f32, kind="ExternalInput")
    sq_d = nc.dram_tensor("sq", [1, KD * BL], f32, kind="ExternalOutput")

    # split the pair tensor between the two DMA queues at a layer-ish
    # boundary so streaming isn't serialized behind one queue
    NPA = (V2_NP + 1) // 2

    trace_sim = bool(int(os.environ.get("MAF_TRACE_SIM", "0")))
    with tile.TileContext(nc, trace_sim=trace_sim) as tc:
        with (
            tc.tile_pool(name="w01", bufs=2) as wpool,
            tc.tile_pool(name="hf", bufs=2) as hpool,
            tc.tile_pool(name="zp", bufs=2) as zpool,
            tc.tile_pool(name="loc", bufs=2) as locpool,
            tc.tile_pool(name="cst", bufs=1) as cpool,
            tc.tile_pool(name="ps", bufs=1, space=bass.MemorySpace.PSUM) as pspool,
        ):
            ones = cpool.tile([P, 1], f32, tag="ones")
            nc.gpsimd.memset(ones[:], 1.0)
            bias_t = cpool.tile([P, STEPS, 56], f32, tag="bias")
            nc.sync.dma_start(bias_t[:], b_d.rearrange("s p k -> p s k"))

            zT = zpool.tile([P, KD, BL], fp8, tag="zT")
            nc.sync.dma_start(zT[:], xt_d[:])
            zloc = zpool.tile([P, KD, BL], f32, tag="zloc")
            nc.sync.dma_start(zloc[:], xloc_d[:])

            for it in range(STEPS * repeat):
                s = it % STEPS
                wA = wpool.tile([P, NPA, 2, P], fp8, tag="wA")
                nc.sync.dma_start(wA[:], wp_d[s, :, :NPA])
                wB = wpool.tile([P, V2_NP - NPA, 2, P], fp8, tag="wB")
                nc.gpsimd.dma_start(wB[:], wp_d[s, :, NPA:])
                wS = wpool.tile([P, V2_NS, P], fp8, tag="wS")
                nc.scalar.dma_start(wS[:], ws_d[s])

                src_of = [zT, None, None, None]
                bcol_of = [0, 16, 32, 48]
                ip = 0
                isg = 0
                h_out = None
                shf = locpool.tile([P, KD, BL], f32, tag="shf")
                cur_ly = -1
                for ly, v, sl in _dp8v2_slots(s % 2):
                    if ly != cur_ly:
                        cur_ly = ly
                        if ly in (1, 2, 3):
                            src = h_out
                        else:
                            src = zT
                        if ly < 3:
                            h_out = hpool.tile([P, 16, BL], fp8, tag=f"h{ly}T")
                    pb = pspool.tile([P, BL], f32, tag=f"pb{v % 8}")
                    nsl = len(sl)
                    for j, (kind, k0) in enumerate(sl):
                        st = (j == 0)
                        sp = (j == nsl - 1)
                        if kind == "p":
                            wt = wA[:, ip, :, :] if ip < NPA else \
                                wB[:, ip - NPA, :, :]
                            nc.tensor.matmul(pb[:], wt, src[:, k0:k0 + 2, :],
                                             start=st, stop=sp, perf_mode=DR)
                            ip += 1
                        else:
                            nc.tensor.matmul(pb[:], wS[:, isg, :],
                                             src[:, k0, :],
                                             start=st, stop=sp)
                            isg += 1
                    if ly < 3:
                        nc.scalar.activation(h_out[:, v, :], pb[:], Relu,
                                             bias=bias_t[:, s,
                                                         bcol_of[ly] + v:
                                                         bcol_of[ly] + v + 1],
                                             scale=1.0 / SW)
                    else:
                        nc.vector.tensor_scalar(shf[:, v, :], pb[:], 1.0 / SW,
                                                bias_t[:, s, 48 + v:48 + v + 1],
                                                mybir.AluOpType.mult,
                                                mybir.AluOpType.add)
                zloc2 = zpool.tile([P, KD, BL], f32, tag="zloc")
                nc.vector.tensor_sub(zloc2[:], zloc[:], shf[:])
                if it != STEPS * repeat - 1:
                    zT = zpool.tile([P, KD, BL], fp8, tag="zT")
                    nc.vector.tensor_sub(zT[:], zloc[:], shf[:])
                zloc = zloc2

            z2 = locpool.tile([P, KD, BL], f32, tag="z2")
            nc.scalar.activation(z2[:], zloc[:], Square, scale=1.0 / SA)
            psq = pspool.tile([1, KD * BL], f32, tag="pb0")
            nc.tensor.matmul(psq[:], ones[:], z2[:].opt(), start=True,
                             stop=True)
            sq_sb = locpool.tile([1, KD * BL], f32, tag="sqsb")
            nc.vector.tensor_copy(sq_sb[:], psq[:])
            nc.sync.dma_start(sq_d[:], sq_sb[:])

    nc.compile()
    return nc

def _dp8_l0_slots(parity):
    """[(m_tile, [(k0, k1), ...])] for the full W0 of one step."""
    out = []
    for v in range(16):
        q = (v // 2 + 1 + 1) // 2 if False else -(-(v // 2 + 1) // 2)
        if parity == 0:
            prs = list(range(q))
        else:
            prs = [3 - t for t in range(q)]
        out.append((v, [(2 * p, 2 * p + 1) for p in prs]))
    return out


def _dp8_l1_slots():
    out = []
    for v in range(16):
        q = -(-(v + 1) // 2)
        out.append((v, [(2 * t, 2 * t + 1) for t in range(q)]))
    return out


def _dp8_l3_slots(parity):
    out = []
    for v in range(8):
        n = (v + 1) if parity == 0 else (8 - v)
        out.append((v, [(2 * t, 2 * t + 1) for t in range(n)]))
    return out


DP8_NS_L0 = sum(len(p) for _, p in _dp8_l0_slots(0))   # 40
DP8_NS_L1 = sum(len(p) for _, p in _dp8_l1_slots())    # 72
DP8_NS_L3 = sum(len(p) for _, p in _dp8_l3_slots(0))   # 36
DP8_NS = DP8_NS_L0 + 2 * DP8_NS_L1 + DP8_NS_L3         # 220


def _prep_inputs_dp8(x, W0, b0, W1, b1, W2, b2, W3, b3):
    from concourse import mybir
    np8 = mybir.dt.np(mybir.dt.float8e4)

    M0 = _made_mask(DIM, H, True)
    M1 = _made_mask(H, H, False)
    M3 = _made_mask(H, DIM, False)

    W0e, W1e, W2e, W3e, b3e = [], [], [], [], []
    for s in range(STEPS):
        w0 = W0[s] * M0
        if s % 2 == 1:
            w0 = w0[::-1, :]
        w3 = W3[s] * M3
        b3s = b3[s]
        if s % 2 == 1:
            w3 = w3[:, ::-1]
            b3s = b3s[::-1]
        W0e.append(np.ascontiguousarray(w0))
        W1e.append(W1[s] * M1)
        W2e.append(W2[s] * M1)
        W3e.append(np.ascontiguousarray(w3))
        b3e.append(np.ascontiguousarray(b3s))

    wall = np.zeros((STEPS, P, DP8_NS, 2, P), dtype=np8)
    bc = np.zeros((STEPS, P, 56), dtype=F32)
    for s in range(STEPS):
        i = 0
        for v, prs in _dp8_l0_slots(s % 2):
            cols = slice(P * v, P * (v + 1))
            for (k0, k1) in prs:
                wall[s, :, i, 0, :] = _q8(W0e[s][P * k0:P * (k0 + 1), cols], np8)
                wall[s, :, i, 1, :] = _q8(W0e[s][P * k1:P * (k1 + 1), cols], np8)
                i += 1
        for We in (W1e, W2e):
            for v, prs in _dp8_l1_slots():
                cols = slice(P * v, P * (v + 1))
                for (k0, k1) in prs:
                    wall[s, :, i, 0, :] = _q8(We[s][P * k0:P * (k0 + 1), cols], np8)
                    wall[s, :, i, 1, :] = _q8(We[s][P * k1:P * (k1 + 1), cols], np8)
                    i += 1
        for v, prs in _dp8_l3_slots(s % 2):
            cols = slice(P * v, P * (v + 1))
            for (k0, k1) in prs:
                wall[s, :, i, 0, :] = _q8(W3e[s][P * k0:P * (k0 + 1), cols], np8)
                wall[s, :, i, 1, :] = _q8(W3e[s][P * k1:P * (k1 + 1), cols], np8)
                i += 1
        assert i == DP8_NS
        for v in range(16):
            bc[s, :, v] = b0[s][P * v:P * (v + 1)] * SA
            bc[s, :, 16 + v] = b1[s][P * v:P * (v + 1)] * SA
            bc[s, :, 32 + v] = b2[s][P * v:P * (v + 1)] * SA
        for v in range(8):
            bc[s, :, 48 + v] = b3e[s][P * v:P * (v + 1)] * SA
    wall = np.ascontiguousarray(wall)
    bc = np.ascontiguousarray(bc)

    xp = np.zeros((NC * BL, DIM), dtype=F32)
    xp[:B] = x * SA
    in_maps = []
    for c in range(NC):
        xc = xp[BL * c:BL * (c + 1)].T          # [DIM, BL]
        zt = xc.reshape(KD, P, BL).transpose(1, 0, 2)   # [P, KD, BL]
        in_maps.append({
            "xloc": np.ascontiguousarray(zt),
            "xt": np.ascontiguousarray(
                np.clip(zt, -FP8_MAX, FP8_MAX).astype(np8)),
            "w": wall,
            "bias": bc,
        })
    return in_maps


def _build_module_dp8(repeat=1):
    from concourse import bass, bacc, tile, mybir

    f32 = mybir.dt.float32
    fp8 = mybir.dt.float8e4
    Relu = mybir.ActivationFunctionType.Relu
    Square = mybir.ActivationFunctionType.Square
    DR = mybir.MatmulPerfMode.DoubleRow

    nc = bacc.Bacc("TRN2", target_bir_lowering=False, debug=False,
                   num_devices=NC)

    NA = DP8_NS_L0 + DP8_NS_L1               # chunk A: L0 + L1 (112)
    xt_d = nc.dram_tensor("xt", [P, KD, BL], fp8, kind="ExternalInput")
    xloc_d = nc.dram_tensor("xloc", [P, KD, BL], f32, kind="ExternalInput")
    w_d = nc.dram_tensor("w", [STEPS, P, DP8_NS, 2, P], fp8,
                         kind="ExternalInput")
    b_d = nc.dram_tensor("bias", [STEPS, P, 56], f32, kind="ExternalInput")
    sq_d = nc.dram_tensor("sq", [1, KD * BL], f32, kind="ExternalOutput")

    trace_sim = bool(int(os.environ.get("MAF_TRACE_SIM", "0")))
    with tile.TileContext(nc, trace_sim=trace_sim) as tc:
        with (
            tc.tile_pool(name="w01", bufs=2) as wpool,
            tc.tile_pool(name="hf", bufs=2) as hpool,
            tc.tile_pool(name="zp", bufs=2) as zpool,
            tc.tile_pool(name="loc", bufs=2) as locpool,
            tc.tile_pool(name="cst", bufs=1) as cpool,
            tc.tile_pool(name="ps", bufs=1, space=bass.MemorySpace.PSUM) as pspool,
        ):
            ones = cpool.tile([P, 1], f32, tag="ones")
            nc.gpsimd.memset(ones[:], 1.0)
            bias_t = cpool.tile([P, STEPS, 56], f32, tag="bias")
            nc.sync.dma_start(bias_t[:], b_d.rearrange("s p k -> p s k"))

            zT = zpool.tile([P, KD, BL], fp8, tag="zT")
            nc.sync.dma_start(zT[:], xt_d[:])
            zloc = zpool.tile([P, KD, BL], f32, tag="zloc")
            nc.sync.dma_start(zloc[:], xloc_d[:])

            l1slots = _dp8_l1_slots()

            for it in range(STEPS * repeat):
                s = it % STEPS
                wA = wpool.tile([P, NA, 2, P], fp8, tag="wA")
                nc.sync.dma_start(wA[:], w_d[s, :, :NA])
                wB = wpool.tile([P, DP8_NS - NA, 2, P], fp8, tag="wB")
                nc.gpsimd.dma_start(wB[:], w_d[s, :, NA:])

                def zpair(k0):
                    return zT[:, k0:k0 + 2, :]

                # L0
                h0T = hpool.tile([P, 16, BL], fp8, tag="h0T")
                i = 0
                for v, prs in _dp8_l0_slots(s % 2):
                    pb = pspool.tile([P, BL], f32, tag=f"pb{v % 8}")
                    for j, (k0, k1) in enumerate(prs):
                        nc.tensor.matmul(pb[:], wA[:, i, :, :], zpair(k0),
                                         start=(j == 0),
                                         stop=(j == len(prs) - 1),
                                         perf_mode=DR)
                        i += 1
                    nc.scalar.activation(h0T[:, v, :], pb[:], Relu,
                                         bias=bias_t[:, s, v:v + 1],
                                         scale=1.0 / SW)

                def h_layer(src, wtile, base, bcol, tag):
                    out = hpool.tile([P, 16, BL], fp8, tag=tag)
                    i = base
                    for v, prs in l1slots:
                        pb = pspool.tile([P, BL], f32, tag=f"pb{v % 8}")
                        for j, (k0, k1) in enumerate(prs):
                            nc.tensor.matmul(pb[:], wtile[:, i, :, :],
                                             src[:, k0:k0 + 2, :],
                                             start=(j == 0),
                                             stop=(j == len(prs) - 1),
                                             perf_mode=DR)
                            i += 1
                        nc.scalar.activation(out[:, v, :], pb[:], Relu,
                                             bias=bias_t[:, s,
                                                         bcol + v:bcol + v + 1],
                                             scale=1.0 / SW)
                    return out

                h1T = h_layer(h0T, wA, DP8_NS_L0, 16, "h1T")
                h2T = h_layer(h1T, wB, 0, 32, "h2T")

                # L3 + z update
                shf = locpool.tile([P, KD, BL], f32, tag="shf")
                i = DP8_NS_L1
                for v, prs in _dp8_l3_slots(s % 2):
                    pb = pspool.tile([P, BL], f32, tag=f"pb{v % 8}")
                    for j, (k0, k1) in enumerate(prs):
                        nc.tensor.matmul(pb[:], wB[:, i, :, :],
                                         h2T[:, k0:k0 + 2, :],
                                         start=(j == 0),
                                         stop=(j == len(prs) - 1),
                                         perf_mode=DR)
                        i += 1
                    nc.vector.tensor_scalar(shf[:, v, :], pb[:], 1.0 / SW,
                                            bias_t[:, s, 48 + v:48 + v + 1],
                                            mybir.AluOpType.mult,
                                            mybir.AluOpType.add)
                zloc2 = zpool.tile([P, KD, BL], f32, tag="zloc")
                nc.vector.tensor_sub(zloc2[:], zloc[:], shf[:])
                if it != STEPS * repeat - 1:
                    zT = zpool.tile([P, KD, BL], fp8, tag="zT")
                    nc.vector.tensor_sub(zT[:], zloc[:], shf[:])
                zloc = zloc2

            z2 = locpool.tile([P, KD, BL], f32, tag="z2")
            nc.scalar.activation(z2[:], zloc[:], Square, scale=1.0 / SA)
            psq = pspool.tile([1, KD * BL], f32, tag="pb0")
            nc.tensor.matmul(psq[:], ones[:], z2[:].opt(), start=True,
                             stop=True)
            sq_sb = locpool.tile([1, KD * BL], f32, tag="sqsb")
            nc.vector.tensor_copy(sq_sb[:], psq[:])
            nc.sync.dma_start(sq_d[:], sq_sb[:])

    nc.compile()
    return nc


_CACHED_NC = {}


IMPL = os.environ.get("MAF_IMPL", "dp8")


def _build_module(repeat=1):
    """Build the SPMD module. repeat>1 runs the whole MAF body N times
    back-to-back (timing builds only; output is then meaningless)."""
    key = (IMPL, repeat)
    if key in _CACHED_NC:
        return _CACHED_NC[key]
    if IMPL == "dp8":
        nc = _build_module_dp8(repeat)
        _CACHED_NC[key] = nc
        return nc

    from concourse import bass, bacc, tile, mybir

    f32 = mybir.dt.float32
    fp8 = mybir.dt.float8e4
    Relu = mybir.ActivationFunctionType.Relu
    Copy = mybir.ActivationFunctionType.Copy
    Square = mybir.ActivationFunctionType.Square
    DR = mybir.MatmulPerfMode.DoubleRow
    RG = [list(range(NC))]
    no_cc = bool(int(os.environ.get("MAF_NO_CC", "0")))  # timing ablation

    nc = bacc.Bacc("TRN2", target_bir_lowering=False, debug=False,
                   num_devices=NC)

    NSLOT = sum(SL0) + 2 * sum(SL1) + SL3  # 6 + 24 + 8 = 38
    xt_d = nc.dram_tensor("xt", [P, KD, B], fp8, kind="ExternalInput")
    xloc_d = nc.dram_tensor("xloc", [P, B], f32, kind="ExternalInput")
    w_d = nc.dram_tensor("w", [STEPS, P, NSLOT, 2, P], fp8,
                         kind="ExternalInput")
    b_d = nc.dram_tensor("bias", [STEPS, P, 7], f32, kind="ExternalInput")
    sq_d = nc.dram_tensor("sq", [1, B], f32, kind="ExternalOutput")

    trace_sim = bool(int(os.environ.get("MAF_TRACE_SIM", "0")))
    with tile.TileContext(nc, trace_sim=trace_sim) as tc:
        with (
            tc.tile_pool(name="w01", bufs=2) as wpool,
            tc.tile_pool(name="hf", bufs=2) as hpool,
            tc.tile_pool(name="zp", bufs=2) as zpool,
            tc.tile_pool(name="loc", bufs=2) as locpool,
            tc.tile_pool(name="cst", bufs=1) as cpool,
            tc.tile_pool(name="ps", bufs=2, space=bass.MemorySpace.PSUM) as pspool,
            tc.tile_pool(name="drb", bufs=2, space="DRAM") as dpool,
        ):
            ones = cpool.tile([P, 1], f32, tag="ones")
            nc.gpsimd.memset(ones[:], 1.0)
            bias_t = cpool.tile([P, STEPS, 7], f32, tag="bias")
            nc.sync.dma_start(bias_t[:], b_d.rearrange("s p k -> p s k"))

            zT = zpool.tile([P, KD, B], fp8, tag="zT")
            nc.sync.dma_start(zT[:], xt_d[:])
            zloc = zpool.tile([P, B], f32, tag="zloc")
            nc.sync.dma_start(zloc[:], xloc_d[:])

            def allgather_h(hloc, out_tag):
                agi = dpool.tile([P, 2, B], fp8, tag="agi")
                # bounce on the scalar queue: same engine as the producing
                # activations, so no cross-engine semaphore hop
                nc.scalar.dma_start(agi[:], hloc[:])
                ago = dpool.tile([4, 2, P, 2, B], fp8, tag="ago")
                if no_cc:
                    nc.sync.dma_start(ago[0, 0, :, :, :], agi[:])
                else:
                    nc.gpsimd.collective_compute(
                        "AllGather", mybir.AluOpType.bypass, replica_groups=RG,
                        ins=[agi.opt()], outs=[ago.opt()])
                hT = hpool.tile([P, 2, 4, 2, B], fp8, tag=out_tag)
                nc.sync.dma_start(hT[:], ago.rearrange("u j p m b -> p m u j b"))
                return hT

            def mm_group(ps, w_t, s, s0, rhs_list, tag):
                n = len(rhs_list)
                for u in range(n):
                    nc.tensor.matmul(ps[:], w_t[:, s, s0 + u, :, :],
                                     rhs_list[u],
                                     start=(u == 0), stop=(u == n - 1),
                                     perf_mode=DR)

            def h_layer(s, w_t, off, bcol, rhsA, rhsB, out_tag):
                hloc = locpool.tile([P, 2, B], fp8, tag="hloc")
                psA = pspool.tile([P, B], f32, tag="psA")
                mm_group(psA, w_t, s, off, rhsA, "A")
                nc.scalar.activation(hloc[:, 0, :], psA[:], Relu,
                                     bias=bias_t[:, s, bcol:bcol + 1],
                                     scale=1.0 / SW)
                psB = pspool.tile([P, B], f32, tag="psB")
                mm_group(psB, w_t, s, off + len(rhsA), rhsB, "B")
                nc.scalar.activation(hloc[:, 1, :], psB[:], Relu,
                                     bias=bias_t[:, s, bcol + 1:bcol + 2],
                                     scale=1.0 / SW)
                return allgather_h(hloc, out_tag)

            wall = None
            for it in range(STEPS * repeat):
                s = it % STEPS
                is_last = it == STEPS * repeat - 1
                if s == 0:
                    # one burst DMA per body: all 4 steps' weights (39 KB
                    # per partition) -> no per-step weight traffic contending
                    # with the AG chain
                    wall = wpool.tile([P, STEPS, NSLOT, 2, P], fp8, tag="w")
                    nc.scalar.dma_start(
                        wall[:], w_d.rearrange("s p n j q -> p s n j q"))
                o1 = sum(SL0)
                o2 = o1 + sum(SL1)
                o3 = o2 + sum(SL1)

                def zpair(t):
                    return zT[:, 2 * t:2 * t + 2, :]

                pa = (0, 1) if s % 2 == 0 else (3, 2)
                h0T = h_layer(s, wall, 0, 0,
                              [zpair(pa[0]), zpair(pa[1])],
                              [zpair(t) for t in range(4)], "h0T")

                def hpair(hT, hg, u):
                    return hT[:, hg, u, :, :]

                rhsA1 = [hpair(h0T, 0, u) for u in range(4)]
                rhsB1 = rhsA1 + [hpair(h0T, 1, u) for u in range(4)]
                h1T = h_layer(s, wall, o1, 2, rhsA1, rhsB1, "h1T")

                rhsA2 = [hpair(h1T, 0, u) for u in range(4)]
                rhsB2 = rhsA2 + [hpair(h1T, 1, u) for u in range(4)]
                h2T = h_layer(s, wall, o2, 4, rhsA2, rhsB2, "h2T")

                rhs3 = [hpair(h2T, 0, u) for u in range(4)] + \
                       [hpair(h2T, 1, u) for u in range(4)]
                ps3 = pspool.tile([P, B], f32, tag="psA")
                mm_group(ps3, wall, s, o3, rhs3, "L3")
                shf = locpool.tile([P, B], f32, tag="shf")
                # shf = ps3/SW + b3*SA (scalars are per-partition APs)
                nc.vector.tensor_scalar(shf[:], ps3[:], 1.0 / SW,
                                        bias_t[:, s, 6:7],
                                        mybir.AluOpType.mult,
                                        mybir.AluOpType.add)
                zloc2 = zpool.tile([P, B], f32, tag="zloc")
                nc.vector.tensor_sub(zloc2[:], zloc[:], shf[:])

                if not is_last:
                    # fp8 copy for the AG computed directly (not serially
                    # after zloc2) so the z critical path has one vector op
                    zlb = locpool.tile([P, B], fp8, tag="zlb")
                    nc.vector.tensor_sub(zlb[:], zloc[:], shf[:])
                    zin = dpool.tile([P, B], fp8, tag="zin")
                    nc.sync.dma_start(zin[:], zlb[:])
                    zout = dpool.tile([NC, P, B], fp8, tag="zout")
                    if no_cc:
                        nc.sync.dma_start(zout[0, :, :], zin[:])
                    else:
                        nc.gpsimd.collective_compute(
                            "AllGather", mybir.AluOpType.bypass,
                            replica_groups=RG,
                            ins=[zin.opt()], outs=[zout.opt()])
                    zT = zpool.tile([P, KD, B], fp8, tag="zT")
                    nc.sync.dma_start(zT[:], zout.rearrange("c p b -> p c b"))
                zloc = zloc2

            z2 = locpool.tile([P, B], f32, tag="z2")
            nc.scalar.activation(z2[:], zloc[:], Square, scale=1.0 / SA)
            psq = pspool.tile([1, B], f32, tag="psq")
            nc.tensor.matmul(psq[:], ones[:], z2[:], start=True, stop=True)
            sq_sb = locpool.tile([1, B], f32, tag="sqsb")
            nc.vector.tensor_copy(sq_sb[:], psq[:])
            nc.sync.dma_start(sq_d[:], sq_sb[:])

    nc.compile()
    _CACHED_NC[(IMPL, repeat)] = nc
    return nc


def kernel(x, W0, b0, W1, b1, W2, b2, W3, b3):
    from concourse import bass_utils

    if IMPL == "dp8":
        in_maps = _prep_inputs_dp8(x, W0, b0, W1, b1, W2, b2, W3, b3)
    else:
        in_maps = _prep_inputs(x, W0, b0, W1, b1, W2, b2, W3, b3)
    nc = _build_module()
    res = bass_utils.run_bass_kernel_spmd(
        nc, in_maps, core_ids=list(range(NC)),
        trace=bool(int(os.environ.get("MAF_TRACE", "0"))))
    if IMPL == "dp8":
        sq = np.concatenate([
            res.results[c]["sq"].reshape(KD, BL).astype(np.float64).sum(0)
            for c in range(NC)])[:B]
        out = 0.5 * sq + 0.5 * DIM * LOG_2PI
    else:
        total = np.zeros(B, dtype=np.float64)
        for c in range(NC):
            total += res.results[c]["sq"][0].astype(np.float64)
        out = 0.5 * total + 0.5 * DIM * LOG_2PI
    if res.exec_time_ns is not None:
        kernel.last_exec_time_ns = res.exec_time_ns
    return out.astype(F32)


kernel.last_exec_time_ns = None



# revision 9
# speedup vs baseline: 1.8704x; 1.8704x over previous
"""Trainium2 Bass kernel for the 4-step shift-only MAF (MADE) chain.

Strategy (v3, IMPL=dp8v2 default): pure data-parallel over the batch across
8 NeuronCores with fully-replicated fp8 weights, masked-tile slot packing
(DoubleRow pairs + single-tile tails), zero collectives. Rationale (measured
on this trn2.8x1 terminal):
  - each ncfw AllGather has a ~6.4us floor + ~4.4us of DRAM-bounce DMA
    hops; the tensor-parallel design needs 15 of them serially -> ~190us
    steady-state, entirely boundary-latency-bound.
  - remote_dma (SBUF->SBUF mesh exchange) crashes this runtime (both the
    remote_dma and proxy gpsimd ucode libraries) - probed and abandoned.
  - per-core HBM->SBUF DMA bandwidth measured 454 GB/s; replicated masked
    weights are 27.3 MB/core -> ~60us streaming floor, which beats every
    collective-bearing variant (>=8 boundaries x ~11us + streaming).
The inter-step `z[:, ::-1]` permute is folded into host-side weight prep.

Older variants kept for A/B: IMPL=tp8 (column-parallel + AllGather),
IMPL=dp8 (data-parallel, pair-only slot packing, 28.8 MB).

v2 changes vs v1 (62.6us / ~194us steady-state):
- fp8 e4m3 weights AND activations with DoubleRow matmuls (256-deep
  contraction per instruction): 4x fewer weight bytes than f32, 2x fewer
  tensor instructions than bf16.
- Block-triangular MADE-mask tile skipping with SPMD-uniform slot padding:
  each core owns column-tiles (c, 15-c) of every hidden layer, so the
  per-core k-pair slot budget is uniform (L0: 2+4, L1/L2: 4+8, L3: 8 of
  dense 4/8/16/8) -> ~75% of dense weight bytes + matmul instructions.
- Weight DMAs split across two DGE queues (scalar + gpsimd) so streaming
  isn't serialized behind one queue; AG bounce copies stay on sync.
- All step biases loaded once up front.

Per-core device program (SPMD; per-core data via in_maps):
  zT [P,8,B] fp8 (z*SA), zloc [P,B] f32 (z*SA of own block) start as x.
  Per step s: L0/L1/L2 column-parallel with Relu(psum/SW + b*SA) -> fp8
  hloc [P,2,B] -> AG -> gathered hT [P,2,4,2,B] (halfgroup, pairidx,
  member, batch). L3 -> shift; zloc -= shift; AG z (not last step).
  Finally sq = ones.T @ (zloc/SA)^2 -> [1,B]; host sums partials.
"""

import os
import sys

import numpy as np

for _p in ("/opt/trn_rl_repo", "/opt/trn_rl_repo/concourse"):
    if _p not in sys.path:
        sys.path.insert(0, _p)

B = 100
DIM = 1024
H = 2048
STEPS = 4
NC = 8
P = 128
KD = DIM // P   # 8 z k-tiles (4 pairs)
KH = H // P     # 16 h k-tiles (8 pairs)
LOG_2PI = float(np.log(2.0 * np.pi))
F32 = np.float32

SW = 64.0   # weight scale into fp8
SA = 16.0   # activation scale into fp8
FP8_MAX = 240.0  # TRN FP8_EXP4 max normal

# slot budgets (pairs of 128-row k-tiles) per layer: [A-slots, B-slots]
SL0 = (2, 4)
SL1 = (4, 8)
SL3 = 8


def _made_mask(n_in, n_out, exclusive):
    d_in, d_out = n_in // DIM, n_out // DIM
    deg_in = np.arange(n_in) // d_in
    deg_out = np.arange(n_out) // d_out
    if exclusive:
        m = deg_out[None, :] > deg_in[:, None]
    else:
        m = deg_out[None, :] >= deg_in[:, None]
    return m.astype(F32)


def _q8(x, np8):
    return np.clip(x * SW, -FP8_MAX, FP8_MAX).astype(np8)


def _l1_slots(c):
    """(slot -> (ktile_j0, ktile_j1)) for the 12 W1/W2 slots of core c."""
    out = []
    for u in range(4):                      # A: pairs 0..3
        out.append((2 * u, 2 * u + 1))
    for u in range(4):                      # B halfgroup 0: pairs 0..3
        out.append((2 * u, 2 * u + 1))
    for u in range(4):                      # B halfgroup 1: pair 7-u
        out.append((15 - 2 * u, 14 - 2 * u))
    return out


def _l0_slots(parity):
    """slot -> (ktile_j0, ktile_j1) for the 6 W0 slots (parity = s%2)."""
    pa = (0, 1) if parity == 0 else (3, 2)
    out = [(2 * p, 2 * p + 1) for p in pa]          # A: 2 slots
    out += [(2 * u, 2 * u + 1) for u in range(4)]   # B: pairs 0..3
    return out


def _l3_slots():
    out = [(2 * u, 2 * u + 1) for u in range(4)]        # halfgroup 0
    out += [(15 - 2 * u, 14 - 2 * u) for u in range(4)]  # halfgroup 1
    return out


def _prep_inputs(x, W0, b0, W1, b1, W2, b2, W3, b3):
    """Host-side: mask, flip-fold, quantize, shard, pack slot layouts."""
    from concourse import mybir
    np8 = mybir.dt.np(mybir.dt.float8e4)

    M0 = _made_mask(DIM, H, True)
    M1 = _made_mask(H, H, False)
    M3 = _made_mask(H, DIM, False)

    xT = np.ascontiguousarray(x.T.astype(F32))              # [1024, 100]
    xts = np.clip(xT * SA, -FP8_MAX, FP8_MAX)
    xt_arr = np.ascontiguousarray(
        xts.reshape(KD, P, B).transpose(1, 0, 2)).astype(np8)  # [128,8,100]

    W0e, W1e, W2e, W3e, b3e = [], [], [], [], []
    for s in range(STEPS):
        w0 = W0[s] * M0
        if s % 2 == 1:
            w0 = w0[::-1, :]
        w3 = W3[s] * M3
        b3s = b3[s]
        if s % 2 == 1:
            w3 = w3[:, ::-1]
            b3s = b3s[::-1]
        W0e.append(np.ascontiguousarray(w0))
        W1e.append(W1[s] * M1)
        W2e.append(W2[s] * M1)
        W3e.append(np.ascontiguousarray(w3))
        b3e.append(np.ascontiguousarray(b3s))

    l1slots = _l1_slots(0)
    l3slots = _l3_slots()

    in_maps = []
    for c in range(NC):
        ca, cb = c, 15 - c                      # col-tile indices
        sa_ = slice(P * ca, P * (ca + 1))
        sb_ = slice(P * cb, P * (cb + 1))

        # W0: [S, P, 6, 2, P]
        w0c = np.zeros((STEPS, P, sum(SL0), 2, P), dtype=np8)
        for s in range(STEPS):
            slots = _l0_slots(s % 2)
            for i, (k0, k1) in enumerate(slots):
                cols = sa_ if i < SL0[0] else sb_
                w0c[s, :, i, 0, :] = _q8(W0e[s][P * k0:P * (k0 + 1), cols], np8)
                w0c[s, :, i, 1, :] = _q8(W0e[s][P * k1:P * (k1 + 1), cols], np8)

        def pack_h(We):
            wc = np.zeros((STEPS, P, sum(SL1), 2, P), dtype=np8)
            for s in range(STEPS):
                for i, (k0, k1) in enumerate(l1slots):
                    cols = sa_ if i < SL1[0] else sb_
                    wc[s, :, i, 0, :] = _q8(We[s][P * k0:P * (k0 + 1), cols], np8)
                    wc[s, :, i, 1, :] = _q8(We[s][P * k1:P * (k1 + 1), cols], np8)
            return wc

        w1c = pack_h(W1e)
        w2c = pack_h(W2e)

        # W3: [S, P, 8, 2, P]; m-tile = z-block c
        zc = slice(P * c, P * (c + 1))
        w3c = np.zeros((STEPS, P, SL3, 2, P), dtype=np8)
        for s in range(STEPS):
            for i, (k0, k1) in enumerate(l3slots):
                w3c[s, :, i, 0, :] = _q8(W3e[s][P * k0:P * (k0 + 1), zc], np8)
                w3c[s, :, i, 1, :] = _q8(W3e[s][P * k1:P * (k1 + 1), zc], np8)

        # biases: [S, P, 7] f32, cols = b0A,b0B,b1A,b1B,b2A,b2B,b3 (all *SA)
        bc = np.zeros((STEPS, P, 7), dtype=F32)
        for s in range(STEPS):
            bc[s, :, 0] = b0[s][sa_] * SA
            bc[s, :, 1] = b0[s][sb_] * SA
            bc[s, :, 2] = b1[s][sa_] * SA
            bc[s, :, 3] = b1[s][sb_] * SA
            bc[s, :, 4] = b2[s][sa_] * SA
            bc[s, :, 5] = b2[s][sb_] * SA
            bc[s, :, 6] = b3e[s][zc] * SA

        wall = np.concatenate([w0c, w1c, w2c, w3c], axis=2)  # [S,P,38,2,P]
        in_maps.append({
            "xt": np.ascontiguousarray(xt_arr),
            "xloc": np.ascontiguousarray(xT[zc, :] * SA),
            "w": np.ascontiguousarray(wall),
            "bias": np.ascontiguousarray(bc),
        })
    return in_maps




# ---------------- DP8 (pure data-parallel) variant ----------------
BL = 16           # padded per-core batch (8 * 16 = 128 >= 100)


# ---- dp8v2: tight mixed pair/single slot packing ----
def _v2_layer_slots(n_ktiles_of_v, ncols, from_top=False, nk=None):
    """Per col-tile v: list of ('p', k0) DoubleRow pairs / ('s', k) singles."""
    out = []
    for v in range(ncols):
        n = n_ktiles_of_v(v)
        slots = []
        if not from_top:
            k = 0
            while n - k >= 2:
                slots.append(("p", k))
                k += 2
            if k < n:
                slots.append(("s", k))
        else:
            hi = nk
            lo = nk - n
            k = hi
            while k - lo >= 2:
                slots.append(("p", k - 2))
                k -= 2
            if k > lo:
                slots.append(("s", lo))
        out.append((v, slots))
    return out


def _dp8v2_slots(parity):
    """[(layer, v, slots)] for one step. layer in 0..3."""
    table = []
    # L0: col v needs n = v//2+1 ktiles; parity 1 flip-folded -> top tiles
    l0 = _v2_layer_slots(lambda v: v // 2 + 1, 16, from_top=(parity == 1), nk=8)
    for v, sl in l0:
        table.append((0, v, sl))
    # L1/L2: col v needs ktiles 0..v
    l1 = _v2_layer_slots(lambda v: v + 1, 16)
    for ly in (1, 2):
        for v, sl in l1:
            table.append((ly, v, sl))
    # L3: parity0 col v needs ktiles 0..2v+1; parity1 col v = orig 7-v
    def l3n(v):
        vv = v if parity == 0 else v  # cols already flip-folded host-side
        return 2 * vv + 2 if parity == 0 else 2 * (7 - vv) + 2
    l3 = _v2_layer_slots(l3n, 8)
    for v, sl in l3:
        table.append((3, v, sl))
    return table


def _dp8v2_counts(parity):
    np_, ns_ = 0, 0
    for _, _, sl in _dp8v2_slots(parity):
        for kind, _ in sl:
            if kind == "p":
                np_ += 1
            else:
                ns_ += 1
    return np_, ns_


V2_NP, V2_NS = _dp8v2_counts(0)     # pairs, singles per step (parity-invt)
assert (V2_NP, V2_NS) == _dp8v2_counts(1), (_dp8v2_counts(0), _dp8v2_counts(1))


def _prep_inputs_dp8v2(x, W0, b0, W1, b1, W2, b2, W3, b3):
    from concourse import mybir
    np8 = mybir.dt.np(mybir.dt.float8e4)

    M0 = _made_mask(DIM, H, True)
    M1 = _made_mask(H, H, False)
    M3 = _made_mask(H, DIM, False)

    W0e, W1e, W2e, W3e, b3e = [], [], [], [], []
    for s in range(STEPS):
        w0 = W0[s] * M0
        if s % 2 == 1:
            w0 = w0[::-1, :]
        w3 = W3[s] * M3
        b3s = b3[s]
        if s % 2 == 1:
            w3 = w3[:, ::-1]
            b3s = b3s[::-1]
        W0e.append(np.ascontiguousarray(w0))
        W1e.append(W1[s] * M1)
        W2e.append(W2[s] * M1)
        W3e.append(np.ascontiguousarray(w3))
        b3e.append(np.ascontiguousarray(b3s))

    wp = np.zeros((STEPS, P, V2_NP, 2, P), dtype=np8)
    ws = np.zeros((STEPS, P, V2_NS, P), dtype=np8)
    bc = np.zeros((STEPS, P, 56), dtype=F32)
    for s in range(STEPS):
        Wl = [W0e[s], W1e[s], W2e[s], W3e[s]]
        ip = 0
        isg = 0
        for ly, v, sl in _dp8v2_slots(s % 2):
            We = Wl[ly]
            cols = slice(P * v, P * (v + 1))
            for kind, k0 in sl:
                if kind == "p":
                    wp[s, :, ip, 0, :] = _q8(We[P * k0:P * (k0 + 1), cols], np8)
                    wp[s, :, ip, 1, :] = _q8(We[P * (k0 + 1):P * (k0 + 2), cols],
                                             np8)
                    ip += 1
                else:
                    ws[s, :, isg, :] = _q8(We[P * k0:P * (k0 + 1), cols], np8)
                    isg += 1
        assert ip == V2_NP and isg == V2_NS, (ip, isg)
        for v in range(16):
            bc[s, :, v] = b0[s][P * v:P * (v + 1)] * SA
            bc[s, :, 16 + v] = b1[s][P * v:P * (v + 1)] * SA
            bc[s, :, 32 + v] = b2[s][P * v:P * (v + 1)] * SA
        for v in range(8):
            bc[s, :, 48 + v] = b3e[s][P * v:P * (v + 1)] * SA
    wp = np.ascontiguousarray(wp)
    ws = np.ascontiguousarray(ws)
    bc = np.ascontiguousarray(bc)

    xp = np.zeros((NC * BL, DIM), dtype=F32)
    xp[:B] = x * SA
    in_maps = []
    for c in range(NC):
        xc = xp[BL * c:BL * (c + 1)].T          # [DIM, BL]
        zt = xc.reshape(KD, P, BL).transpose(1, 0, 2)   # [P, KD, BL]
        in_maps.append({
            "xloc": np.ascontiguousarray(zt),
            "xt": np.ascontiguousarray(
                np.clip(zt, -FP8_MAX, FP8_MAX).astype(np8)),
            "wp": wp,
            "ws": ws,
            "bias": bc,
        })
    return in_maps


def _build_module_dp8v2(repeat=1):
    from concourse import bass, bacc, tile, mybir

    f32 = mybir.dt.float32
    fp8 = mybir.dt.float8e4
    Relu = mybir.ActivationFunctionType.Relu
    Square = mybir.ActivationFunctionType.Square
    DR = mybir.MatmulPerfMode.DoubleRow

    nc = bacc.Bacc("TRN2", target_bir_lowering=False, debug=False,
                   num_devices=NC)

    xt_d = nc.dram_tensor("xt", [P, KD, BL], fp8, kind="ExternalInput")
    xloc_d = nc.dram_tensor("xloc", [P, KD, BL], f32, kind="ExternalInput")
    wp_d = nc.dram_tensor("wp", [STEPS, P, V2_NP, 2, P], fp8,
                          kind="ExternalInput")
    ws_d = nc.dram_tensor("ws", [STEPS, P, V2_NS, P], fp8,
                          kind="ExternalInput")
    b_d = nc.dram_tensor("bias", [STEPS, P, 56], f32, kind="ExternalInput")
    sq_d = nc.dram_tensor("sq", [1, KD * BL], f32, kind="ExternalOutput")

    # weight-DMA queue layout: "split" = pairs across sync+gpsimd, singles
    # on scalar; "one" = everything on sync (HWDGE)
    WQ = os.environ.get("MAF_WQ", "split")
    NPA = (V2_NP + 1) // 2 if WQ == "split" else V2_NP

    trace_sim = bool(int(os.environ.get("MAF_TRACE_SIM", "0")))
    with tile.TileContext(nc, trace_sim=trace_sim) as tc:
        with (
            tc.tile_pool(name="w01", bufs=2) as wpool,
            tc.tile_pool(name="hf", bufs=2) as hpool,
            tc.tile_pool(name="zp", bufs=2) as zpool,
            tc.tile_pool(name="loc", bufs=2) as locpool,
            tc.tile_pool(name="cst", bufs=1) as cpool,
            tc.tile_pool(name="ps", bufs=1, space=bass.MemorySpace.PSUM) as pspool,
        ):
            ones = cpool.tile([P, 1], f32, tag="ones")
            nc.gpsimd.memset(ones[:], 1.0)
            bias_t = cpool.tile([P, STEPS, 56], f32, tag="bias")
            nc.sync.dma_start(bias_t[:], b_d.rearrange("s p k -> p s k"))

            zT = zpool.tile([P, KD, BL], fp8, tag="zT")
            nc.sync.dma_start(zT[:], xt_d[:])
            zloc = zpool.tile([P, KD, BL], f32, tag="zloc")
            nc.sync.dma_start(zloc[:], xloc_d[:])

            for it in range(STEPS * repeat):
                s = it % STEPS
                wA = wpool.tile([P, NPA, 2, P], fp8, tag="wA")
                nc.sync.dma_start(wA[:], wp_d[s, :, :NPA])
                if WQ == "split":
                    wB = wpool.tile([P, V2_NP - NPA, 2, P], fp8, tag="wB")
                    nc.gpsimd.dma_start(wB[:], wp_d[s, :, NPA:])
                else:
                    wB = None
                wS = wpool.tile([P, V2_NS, P], fp8, tag="wS")
                (nc.scalar if WQ == "split" else nc.sync).dma_start(
                    wS[:], ws_d[s])

                src_of = [zT, None, None, None]
                bcol_of = [0, 16, 32, 48]
                ip = 0
                isg = 0
                h_out = None
                shf = locpool.tile([P, KD, BL], f32, tag="shf")
                cur_ly = -1
                for ly, v, sl in _dp8v2_slots(s % 2):
                    if ly != cur_ly:
                        cur_ly = ly
                        if ly in (1, 2, 3):
                            src = h_out
                        else:
                            src = zT
                        if ly < 3:
                            h_out = hpool.tile([P, 16, BL], fp8, tag=f"h{ly}T")
                    pb = pspool.tile([P, BL], f32, tag=f"pb{v % 8}")
                    nsl = len(sl)
                    for j, (kind, k0) in enumerate(sl):
                        st = (j == 0)
                        sp = (j == nsl - 1)
                        if kind == "p":
                            wt = wA[:, ip, :, :] if ip < NPA else \
                                wB[:, ip - NPA, :, :]
                            nc.tensor.matmul(pb[:], wt, src[:, k0:k0 + 2, :],
                                             start=st, stop=sp, perf_mode=DR)
                            ip += 1
                        else:
                            nc.tensor.matmul(pb[:], wS[:, isg, :],
                                             src[:, k0, :],
                                             start=st, stop=sp)
                            isg += 1
                    if ly < 3:
                        nc.scalar.activation(h_out[:, v, :], pb[:], Relu,
                                             bias=bias_t[:, s,
                                                         bcol_of[ly] + v:
                                                         bcol_of[ly] + v + 1],
                                             scale=1.0 / SW)
                    else:
                        nc.vector.tensor_scalar(shf[:, v, :], pb[:], 1.0 / SW,
                                                bias_t[:, s, 48 + v:48 + v + 1],
                                                mybir.AluOpType.mult,
                                                mybir.AluOpType.add)
                zloc2 = zpool.tile([P, KD, BL], f32, tag="zloc")
                nc.vector.tensor_sub(zloc2[:], zloc[:], shf[:])
                if it != STEPS * repeat - 1:
                    zT = zpool.tile([P, KD, BL], fp8, tag="zT")
                    nc.vector.tensor_sub(zT[:], zloc[:], shf[:])
                zloc = zloc2

            z2 = locpool.tile([P, KD, BL], f32, tag="z2")
            nc.scalar.activation(z2[:], zloc[:], Square, scale=1.0 / SA)
            psq = pspool.tile([1, KD * BL], f32, tag="pb0")
            nc.tensor.matmul(psq[:], ones[:], z2[:].opt(), start=True,
                             stop=True)
            sq_sb = locpool.tile([1, KD * BL], f32, tag="sqsb")
            nc.vector.tensor_copy(sq_sb[:], psq[:])
            nc.sync.dma_start(sq_d[:], sq_sb[:])

    nc.compile()
    return nc

def _dp8_l0_slots(parity):
    """[(m_tile, [(k0, k1), ...])] for the full W0 of one step."""
    out = []
    for v in range(16):
        q = (v // 2 + 1 + 1) // 2 if False else -(-(v // 2 + 1) // 2)
        if parity == 0:
            prs = list(range(q))
        else:
            prs = [3 - t for t in range(q)]
        out.append((v, [(2 * p, 2 * p + 1) for p in prs]))
    return out


def _dp8_l1_slots():
    out = []
    for v in range(16):
        q = -(-(v + 1) // 2)
        out.append((v, [(2 * t, 2 * t + 1) for t in range(q)]))
    return out


def _dp8_l3_slots(parity):
    out = []
    for v in range(8):
        n = (v + 1) if parity == 0 else (8 - v)
        out.append((v, [(2 * t, 2 * t + 1) for t in range(n)]))
    return out


DP8_NS_L0 = sum(len(p) for _, p in _dp8_l0_slots(0))   # 40
DP8_NS_L1 = sum(len(p) for _, p in _dp8_l1_slots())    # 72
DP8_NS_L3 = sum(len(p) for _, p in _dp8_l3_slots(0))   # 36
DP8_NS = DP8_NS_L0 + 2 * DP8_NS_L1 + DP8_NS_L3         # 220


def _prep_inputs_dp8(x, W0, b0, W1, b1, W2, b2, W3, b3):
    from concourse import mybir
    np8 = mybir.dt.np(mybir.dt.float8e4)

    M0 = _made_mask(DIM, H, True)
    M1 = _made_mask(H, H, False)
    M3 = _made_mask(H, DIM, False)

    W0e, W1e, W2e, W3e, b3e = [], [], [], [], []
    for s in range(STEPS):
        w0 = W0[s] * M0
        if s % 2 == 1:
            w0 = w0[::-1, :]
        w3 = W3[s] * M3
        b3s = b3[s]
        if s % 2 == 1:
            w3 = w3[:, ::-1]
            b3s = b3s[::-1]
        W0e.append(np.ascontiguousarray(w0))
        W1e.append(W1[s] * M1)
        W2e.append(W2[s] * M1)
        W3e.append(np.ascontiguousarray(w3))
        b3e.append(np.ascontiguousarray(b3s))

    wall = np.zeros((STEPS, P, DP8_NS, 2, P), dtype=np8)
    bc = np.zeros((STEPS, P, 56), dtype=F32)
    for s in range(STEPS):
        i = 0
        for v, prs in _dp8_l0_slots(s % 2):
            cols = slice(P * v, P * (v + 1))
            for (k0, k1) in prs:
                wall[s, :, i, 0, :] = _q8(W0e[s][P * k0:P * (k0 + 1), cols], np8)
                wall[s, :, i, 1, :] = _q8(W0e[s][P * k1:P * (k1 + 1), cols], np8)
                i += 1
        for We in (W1e, W2e):
            for v, prs in _dp8_l1_slots():
                cols = slice(P * v, P * (v + 1))
                for (k0, k1) in prs:
                    wall[s, :, i, 0, :] = _q8(We[s][P * k0:P * (k0 + 1), cols], np8)
                    wall[s, :, i, 1, :] = _q8(We[s][P * k1:P * (k1 + 1), cols], np8)
                    i += 1
        for v, prs in _dp8_l3_slots(s % 2):
            cols = slice(P * v, P * (v + 1))
            for (k0, k1) in prs:
                wall[s, :, i, 0, :] = _q8(W3e[s][P * k0:P * (k0 + 1), cols], np8)
                wall[s, :, i, 1, :] = _q8(W3e[s][P * k1:P * (k1 + 1), cols], np8)
                i += 1
        assert i == DP8_NS
        for v in range(16):
            bc[s, :, v] = b0[s][P * v:P * (v + 1)] * SA
            bc[s, :, 16 + v] = b1[s][P * v:P * (v + 1)] * SA
            bc[s, :, 32 + v] = b2[s][P * v:P * (v + 1)] * SA
        for v in range(8):
            bc[s, :, 48 + v] = b3e[s][P * v:P * (v + 1)] * SA
    wall = np.ascontiguousarray(wall)
    bc = np.ascontiguousarray(bc)

    xp = np.zeros((NC * BL, DIM), dtype=F32)
    xp[:B] = x * SA
    in_maps = []
    for c in range(NC):
        xc = xp[BL * c:BL * (c + 1)].T          # [DIM, BL]
        zt = xc.reshape(KD, P, BL).transpose(1, 0, 2)   # [P, KD, BL]
        in_maps.append({
            "xloc": np.ascontiguousarray(zt),
            "xt": np.ascontiguousarray(
                np.clip(zt, -FP8_MAX, FP8_MAX).astype(np8)),
            "w": wall,
            "bias": bc,
        })
    return in_maps


def _build_module_dp8(repeat=1):
    from concourse import bass, bacc, tile, mybir

    f32 = mybir.dt.float32
    fp8 = mybir.dt.float8e4
    Relu = mybir.ActivationFunctionType.Relu
    Square = mybir.ActivationFunctionType.Square
    DR = mybir.MatmulPerfMode.DoubleRow

    nc = bacc.Bacc("TRN2", target_bir_lowering=False, debug=False,
                   num_devices=NC)

    NA = DP8_NS_L0 + DP8_NS_L1               # chunk A: L0 + L1 (112)
    xt_d = nc.dram_tensor("xt", [P, KD, BL], fp8, kind="ExternalInput")
    xloc_d = nc.dram_tensor("xloc", [P, KD, BL], f32, kind="ExternalInput")
    w_d = nc.dram_tensor("w", [STEPS, P, DP8_NS, 2, P], fp8,
                         kind="ExternalInput")
    b_d = nc.dram_tensor("bias", [STEPS, P, 56], f32, kind="ExternalInput")
    sq_d = nc.dram_tensor("sq", [1, KD * BL], f32, kind="ExternalOutput")

    trace_sim = bool(int(os.environ.get("MAF_TRACE_SIM", "0")))
    with tile.TileContext(nc, trace_sim=trace_sim) as tc:
        with (
            tc.tile_pool(name="w01", bufs=2) as wpool,
            tc.tile_pool(name="hf", bufs=2) as hpool,
            tc.tile_pool(name="zp", bufs=2) as zpool,
            tc.tile_pool(name="loc", bufs=2) as locpool,
            tc.tile_pool(name="cst", bufs=1) as cpool,
            tc.tile_pool(name="ps", bufs=1, space=bass.MemorySpace.PSUM) as pspool,
        ):
            ones = cpool.tile([P, 1], f32, tag="ones")
            nc.gpsimd.memset(ones[:], 1.0)
            bias_t = cpool.tile([P, STEPS, 56], f32, tag="bias")
            nc.sync.dma_start(bias_t[:], b_d.rearrange("s p k -> p s k"))

            zT = zpool.tile([P, KD, BL], fp8, tag="zT")
            nc.sync.dma_start(zT[:], xt_d[:])
            zloc = zpool.tile([P, KD, BL], f32, tag="zloc")
            nc.sync.dma_start(zloc[:], xloc_d[:])

            l1slots = _dp8_l1_slots()

            for it in range(STEPS * repeat):
                s = it % STEPS
                wA = wpool.tile([P, NA, 2, P], fp8, tag="wA")
                nc.sync.dma_start(wA[:], w_d[s, :, :NA])
                wB = wpool.tile([P, DP8_NS - NA, 2, P], fp8, tag="wB")
                nc.gpsimd.dma_start(wB[:], w_d[s, :, NA:])

                def zpair(k0):
                    return zT[:, k0:k0 + 2, :]

                # L0
                h0T = hpool.tile([P, 16, BL], fp8, tag="h0T")
                i = 0
                for v, prs in _dp8_l0_slots(s % 2):
                    pb = pspool.tile([P, BL], f32, tag=f"pb{v % 8}")
                    for j, (k0, k1) in enumerate(prs):
                        nc.tensor.matmul(pb[:], wA[:, i, :, :], zpair(k0),
                                         start=(j == 0),
                                         stop=(j == len(prs) - 1),
                                         perf_mode=DR)
                        i += 1
                    nc.scalar.activation(h0T[:, v, :], pb[:], Relu,
                                         bias=bias_t[:, s, v:v + 1],
                                         scale=1.0 / SW)

                def h_layer(src, wtile, base, bcol, tag):
                    out = hpool.tile([P, 16, BL], fp8, tag=tag)
                    i = base
                    for v, prs in l1slots:
                        pb = pspool.tile([P, BL], f32, tag=f"pb{v % 8}")
                        for j, (k0, k1) in enumerate(prs):
                            nc.tensor.matmul(pb[:], wtile[:, i, :, :],
                                             src[:, k0:k0 + 2, :],
                                             start=(j == 0),
                                             stop=(j == len(prs) - 1),
                                             perf_mode=DR)
                            i += 1
                        nc.scalar.activation(out[:, v, :], pb[:], Relu,
                                             bias=bias_t[:, s,
                                                         bcol + v:bcol + v + 1],
                                             scale=1.0 / SW)
                    return out

                h1T = h_layer(h0T, wA, DP8_NS_L0, 16, "h1T")
                h2T = h_layer(h1T, wB, 0, 32, "h2T")

                # L3 + z update
                shf = locpool.tile([P, KD, BL], f32, tag="shf")
                i = DP8_NS_L1
                for v, prs in _dp8_l3_slots(s % 2):
                    pb = pspool.tile([P, BL], f32, tag=f"pb{v % 8}")
                    for j, (k0, k1) in enumerate(prs):
                        nc.tensor.matmul(pb[:], wB[:, i, :, :],
                                         h2T[:, k0:k0 + 2, :],
                                         start=(j == 0),
                                         stop=(j == len(prs) - 1),
                                         perf_mode=DR)
                        i += 1
                    nc.vector.tensor_scalar(shf[:, v, :], pb[:], 1.0 / SW,
                                            bias_t[:, s, 48 + v:48 + v + 1],
                                            mybir.AluOpType.mult,
                                            mybir.AluOpType.add)
                zloc2 = zpool.tile([P, KD, BL], f32, tag="zloc")
                nc.vector.tensor_sub(zloc2[:], zloc[:], shf[:])
                if it != STEPS * repeat - 1:
                    zT = zpool.tile([P, KD, BL], fp8, tag="zT")
                    nc.vector.tensor_sub(zT[:], zloc[:], shf[:])
                zloc = zloc2

            z2 = locpool.tile([P, KD, BL], f32, tag="z2")
            nc.scalar.activation(z2[:], zloc[:], Square, scale=1.0 / SA)
            psq = pspool.tile([1, KD * BL], f32, tag="pb0")
            nc.tensor.matmul(psq[:], ones[:], z2[:].opt(), start=True,
                             stop=True)
            sq_sb = locpool.tile([1, KD * BL], f32, tag="sqsb")
            nc.vector.tensor_copy(sq_sb[:], psq[:])
            nc.sync.dma_start(sq_d[:], sq_sb[:])

    nc.compile()
    return nc


_CACHED_NC = {}


IMPL = os.environ.get("MAF_IMPL", "dp8v2")


def _build_module(repeat=1):
    """Build the SPMD module. repeat>1 runs the whole MAF body N times
    back-to-back (timing builds only; output is then meaningless)."""
    key = (IMPL, repeat)
    if key in _CACHED_NC:
        return _CACHED_NC[key]
    if IMPL == "dp8":
        nc = _build_module_dp8(repeat)
        _CACHED_NC[key] = nc
        return nc
    if IMPL == "dp8v2":
        nc = _build_module_dp8v2(repeat)
        _CACHED_NC[key] = nc
        return nc

    from concourse import bass, bacc, tile, mybir

    f32 = mybir.dt.float32
    fp8 = mybir.dt.float8e4
    Relu = mybir.ActivationFunctionType.Relu
    Copy = mybir.ActivationFunctionType.Copy
    Square = mybir.ActivationFunctionType.Square
    DR = mybir.MatmulPerfMode.DoubleRow
    RG = [list(range(NC))]
    no_cc = bool(int(os.environ.get("MAF_NO_CC", "0")))  # timing ablation

    nc = bacc.Bacc("TRN2", target_bir_lowering=False, debug=False,
                   num_devices=NC)

    NSLOT = sum(SL0) + 2 * sum(SL1) + SL3  # 6 + 24 + 8 = 38
    xt_d = nc.dram_tensor("xt", [P, KD, B], fp8, kind="ExternalInput")
    xloc_d = nc.dram_tensor("xloc", [P, B], f32, kind="ExternalInput")
    w_d = nc.dram_tensor("w", [STEPS, P, NSLOT, 2, P], fp8,
                         kind="ExternalInput")
    b_d = nc.dram_tensor("bias", [STEPS, P, 7], f32, kind="ExternalInput")
    sq_d = nc.dram_tensor("sq", [1, B], f32, kind="ExternalOutput")

    trace_sim = bool(int(os.environ.get("MAF_TRACE_SIM", "0")))
    with tile.TileContext(nc, trace_sim=trace_sim) as tc:
        with (
            tc.tile_pool(name="w01", bufs=2) as wpool,
            tc.tile_pool(name="hf", bufs=2) as hpool,
            tc.tile_pool(name="zp", bufs=2) as zpool,
            tc.tile_pool(name="loc", bufs=2) as locpool,
            tc.tile_pool(name="cst", bufs=1) as cpool,
            tc.tile_pool(name="ps", bufs=2, space=bass.MemorySpace.PSUM) as pspool,
            tc.tile_pool(name="drb", bufs=2, space="DRAM") as dpool,
        ):
            ones = cpool.tile([P, 1], f32, tag="ones")
            nc.gpsimd.memset(ones[:], 1.0)
            bias_t = cpool.tile([P, STEPS, 7], f32, tag="bias")
            nc.sync.dma_start(bias_t[:], b_d.rearrange("s p k -> p s k"))

            zT = zpool.tile([P, KD, B], fp8, tag="zT")
            nc.sync.dma_start(zT[:], xt_d[:])
            zloc = zpool.tile([P, B], f32, tag="zloc")
            nc.sync.dma_start(zloc[:], xloc_d[:])

            def allgather_h(hloc, out_tag):
                agi = dpool.tile([P, 2, B], fp8, tag="agi")
                # bounce on the scalar queue: same engine as the producing
                # activations, so no cross-engine semaphore hop
                nc.scalar.dma_start(agi[:], hloc[:])
                ago = dpool.tile([4, 2, P, 2, B], fp8, tag="ago")
                if no_cc:
                    nc.sync.dma_start(ago[0, 0, :, :, :], agi[:])
                else:
                    nc.gpsimd.collective_compute(
                        "AllGather", mybir.AluOpType.bypass, replica_groups=RG,
                        ins=[agi.opt()], outs=[ago.opt()])
                hT = hpool.tile([P, 2, 4, 2, B], fp8, tag=out_tag)
                nc.sync.dma_start(hT[:], ago.rearrange("u j p m b -> p m u j b"))
                return hT

            def mm_group(ps, w_t, s, s0, rhs_list, tag):
                n = len(rhs_list)
                for u in range(n):
                    nc.tensor.matmul(ps[:], w_t[:, s, s0 + u, :, :],
                                     rhs_list[u],
                                     start=(u == 0), stop=(u == n - 1),
                                     perf_mode=DR)

            def h_layer(s, w_t, off, bcol, rhsA, rhsB, out_tag):
                hloc = locpool.tile([P, 2, B], fp8, tag="hloc")
                psA = pspool.tile([P, B], f32, tag="psA")
                mm_group(psA, w_t, s, off, rhsA, "A")
                nc.scalar.activation(hloc[:, 0, :], psA[:], Relu,
                                     bias=bias_t[:, s, bcol:bcol + 1],
                                     scale=1.0 / SW)
                psB = pspool.tile([P, B], f32, tag="psB")
                mm_group(psB, w_t, s, off + len(rhsA), rhsB, "B")
                nc.scalar.activation(hloc[:, 1, :], psB[:], Relu,
                                     bias=bias_t[:, s, bcol + 1:bcol + 2],
                                     scale=1.0 / SW)
                return allgather_h(hloc, out_tag)

            wall = None
            for it in range(STEPS * repeat):
                s = it % STEPS
                is_last = it == STEPS * repeat - 1
                if s == 0:
                    # one burst DMA per body: all 4 steps' weights (39 KB
                    # per partition) -> no per-step weight traffic contending
                    # with the AG chain
                    wall = wpool.tile([P, STEPS, NSLOT, 2, P], fp8, tag="w")
                    nc.scalar.dma_start(
                        wall[:], w_d.rearrange("s p n j q -> p s n j q"))
                o1 = sum(SL0)
                o2 = o1 + sum(SL1)
                o3 = o2 + sum(SL1)

                def zpair(t):
                    return zT[:, 2 * t:2 * t + 2, :]

                pa = (0, 1) if s % 2 == 0 else (3, 2)
                h0T = h_layer(s, wall, 0, 0,
                              [zpair(pa[0]), zpair(pa[1])],
                              [zpair(t) for t in range(4)], "h0T")

                def hpair(hT, hg, u):
                    return hT[:, hg, u, :, :]

                rhsA1 = [hpair(h0T, 0, u) for u in range(4)]
                rhsB1 = rhsA1 + [hpair(h0T, 1, u) for u in range(4)]
                h1T = h_layer(s, wall, o1, 2, rhsA1, rhsB1, "h1T")

                rhsA2 = [hpair(h1T, 0, u) for u in range(4)]
                rhsB2 = rhsA2 + [hpair(h1T, 1, u) for u in range(4)]
                h2T = h_layer(s, wall, o2, 4, rhsA2, rhsB2, "h2T")

                rhs3 = [hpair(h2T, 0, u) for u in range(4)] + \
                       [hpair(h2T, 1, u) for u in range(4)]
                ps3 = pspool.tile([P, B], f32, tag="psA")
                mm_group(ps3, wall, s, o3, rhs3, "L3")
                shf = locpool.tile([P, B], f32, tag="shf")
                # shf = ps3/SW + b3*SA (scalars are per-partition APs)
                nc.vector.tensor_scalar(shf[:], ps3[:], 1.0 / SW,
                                        bias_t[:, s, 6:7],
                                        mybir.AluOpType.mult,
                                        mybir.AluOpType.add)
                zloc2 = zpool.tile([P, B], f32, tag="zloc")
                nc.vector.tensor_sub(zloc2[:], zloc[:], shf[:])

                if not is_last:
                    # fp8 copy for the AG computed directly (not serially
                    # after zloc2) so the z critical path has one vector op
                    zlb = locpool.tile([P, B], fp8, tag="zlb")
                    nc.vector.tensor_sub(zlb[:], zloc[:], shf[:])
                    zin = dpool.tile([P, B], fp8, tag="zin")
                    nc.sync.dma_start(zin[:], zlb[:])
                    zout = dpool.tile([NC, P, B], fp8, tag="zout")
                    if no_cc:
                        nc.sync.dma_start(zout[0, :, :], zin[:])
                    else:
                        nc.gpsimd.collective_compute(
                            "AllGather", mybir.AluOpType.bypass,
                            replica_groups=RG,
                            ins=[zin.opt()], outs=[zout.opt()])
                    zT = zpool.tile([P, KD, B], fp8, tag="zT")
                    nc.sync.dma_start(zT[:], zout.rearrange("c p b -> p c b"))
                zloc = zloc2

            z2 = locpool.tile([P, B], f32, tag="z2")
            nc.scalar.activation(z2[:], zloc[:], Square, scale=1.0 / SA)
            psq = pspool.tile([1, B], f32, tag="psq")
            nc.tensor.matmul(psq[:], ones[:], z2[:], start=True, stop=True)
            sq_sb = locpool.tile([1, B], f32, tag="sqsb")
            nc.vector.tensor_copy(sq_sb[:], psq[:])
            nc.sync.dma_start(sq_d[:], sq_sb[:])

    nc.compile()
    _CACHED_NC[(IMPL, repeat)] = nc
    return nc


def kernel(x, W0, b0, W1, b1, W2, b2, W3, b3):
    from concourse import bass_utils

    if IMPL == "dp8":
        in_maps = _prep_inputs_dp8(x, W0, b0, W1, b1, W2, b2, W3, b3)
    elif IMPL == "dp8v2":
        in_maps = _prep_inputs_dp8v2(x, W0, b0, W1, b1, W2, b2, W3, b3)
    else:
        in_maps = _prep_inputs(x, W0, b0, W1, b1, W2, b2, W3, b3)
    nc = _build_module()
    res = bass_utils.run_bass_kernel_spmd(
        nc, in_maps, core_ids=list(range(NC)),
        trace=bool(int(os.environ.get("MAF_TRACE", "0"))))
    if IMPL in ("dp8", "dp8v2"):
        sq = np.concatenate([
            res.results[c]["sq"].reshape(KD, BL).astype(np.float64).sum(0)
            for c in range(NC)])[:B]
        out = 0.5 * sq + 0.5 * DIM * LOG_2PI
    else:
        total = np.zeros(B, dtype=np.float64)
        for c in range(NC):
            total += res.results[c]["sq"][0].astype(np.float64)
        out = 0.5 * total + 0.5 * DIM * LOG_2PI
    if res.exec_time_ns is not None:
        kernel.last_exec_time_ns = res.exec_time_ns
    return out.astype(F32)


kernel.last_exec_time_ns = None



# revision 15
# speedup vs baseline: 3.2133x; 1.7180x over previous
"""Trainium2 Bass kernel for the 4-step shift-only MAF (MADE) chain.

Strategy (v3, IMPL=dp8v2 default): pure data-parallel over the batch across
8 NeuronCores with fully-replicated fp8 weights, masked-tile slot packing
(DoubleRow pairs + single-tile tails), zero collectives. Rationale (measured
on this trn2.8x1 terminal):
  - each ncfw AllGather has a ~6.4us floor + ~4.4us of DRAM-bounce DMA
    hops; the tensor-parallel design needs 15 of them serially -> ~190us
    steady-state, entirely boundary-latency-bound.
  - remote_dma (SBUF->SBUF mesh exchange) crashes this runtime (both the
    remote_dma and proxy gpsimd ucode libraries) - probed and abandoned.
  - per-core HBM->SBUF DMA bandwidth measured 454 GB/s; replicated masked
    weights are 27.3 MB/core -> ~60us streaming floor, which beats every
    collective-bearing variant (>=8 boundaries x ~11us + streaming).
The inter-step `z[:, ::-1]` permute is folded into host-side weight prep.

Older variants kept for A/B: IMPL=tp8 (column-parallel + AllGather),
IMPL=dp8 (data-parallel, pair-only slot packing, 28.8 MB).

v2 changes vs v1 (62.6us / ~194us steady-state):
- fp8 e4m3 weights AND activations with DoubleRow matmuls (256-deep
  contraction per instruction): 4x fewer weight bytes than f32, 2x fewer
  tensor instructions than bf16.
- Block-triangular MADE-mask tile skipping with SPMD-uniform slot padding:
  each core owns column-tiles (c, 15-c) of every hidden layer, so the
  per-core k-pair slot budget is uniform (L0: 2+4, L1/L2: 4+8, L3: 8 of
  dense 4/8/16/8) -> ~75% of dense weight bytes + matmul instructions.
- Weight DMAs split across two DGE queues (scalar + gpsimd) so streaming
  isn't serialized behind one queue; AG bounce copies stay on sync.
- All step biases loaded once up front.

Per-core device program (SPMD; per-core data via in_maps):
  zT [P,8,B] fp8 (z*SA), zloc [P,B] f32 (z*SA of own block) start as x.
  Per step s: L0/L1/L2 column-parallel with Relu(psum/SW + b*SA) -> fp8
  hloc [P,2,B] -> AG -> gathered hT [P,2,4,2,B] (halfgroup, pairidx,
  member, batch). L3 -> shift; zloc -= shift; AG z (not last step).
  Finally sq = ones.T @ (zloc/SA)^2 -> [1,B]; host sums partials.
"""

import os
import sys

import numpy as np

for _p in ("/opt/trn_rl_repo", "/opt/trn_rl_repo/concourse"):
    if _p not in sys.path:
        sys.path.insert(0, _p)

B = 100
DIM = 1024
H = 2048
STEPS = 4
NC = 8
P = 128
KD = DIM // P   # 8 z k-tiles (4 pairs)
KH = H // P     # 16 h k-tiles (8 pairs)
LOG_2PI = float(np.log(2.0 * np.pi))
F32 = np.float32

SW = 64.0   # weight scale into fp8
SA = 16.0   # activation scale into fp8
FP8_MAX = 240.0  # TRN FP8_EXP4 max normal

# slot budgets (pairs of 128-row k-tiles) per layer: [A-slots, B-slots]
SL0 = (2, 4)
SL1 = (4, 8)
SL3 = 8


def _made_mask(n_in, n_out, exclusive):
    d_in, d_out = n_in // DIM, n_out // DIM
    deg_in = np.arange(n_in) // d_in
    deg_out = np.arange(n_out) // d_out
    if exclusive:
        m = deg_out[None, :] > deg_in[:, None]
    else:
        m = deg_out[None, :] >= deg_in[:, None]
    return m.astype(F32)


def _q8(x, np8):
    return np.clip(x * SW, -FP8_MAX, FP8_MAX).astype(np8)


def _l1_slots(c):
    """(slot -> (ktile_j0, ktile_j1)) for the 12 W1/W2 slots of core c."""
    out = []
    for u in range(4):                      # A: pairs 0..3
        out.append((2 * u, 2 * u + 1))
    for u in range(4):                      # B halfgroup 0: pairs 0..3
        out.append((2 * u, 2 * u + 1))
    for u in range(4):                      # B halfgroup 1: pair 7-u
        out.append((15 - 2 * u, 14 - 2 * u))
    return out


def _l0_slots(parity):
    """slot -> (ktile_j0, ktile_j1) for the 6 W0 slots (parity = s%2)."""
    pa = (0, 1) if parity == 0 else (3, 2)
    out = [(2 * p, 2 * p + 1) for p in pa]          # A: 2 slots
    out += [(2 * u, 2 * u + 1) for u in range(4)]   # B: pairs 0..3
    return out


def _l3_slots():
    out = [(2 * u, 2 * u + 1) for u in range(4)]        # halfgroup 0
    out += [(15 - 2 * u, 14 - 2 * u) for u in range(4)]  # halfgroup 1
    return out


def _prep_inputs(x, W0, b0, W1, b1, W2, b2, W3, b3):
    """Host-side: mask, flip-fold, quantize, shard, pack slot layouts."""
    from concourse import mybir
    np8 = mybir.dt.np(mybir.dt.float8e4)

    M0 = _made_mask(DIM, H, True)
    M1 = _made_mask(H, H, False)
    M3 = _made_mask(H, DIM, False)

    xT = np.ascontiguousarray(x.T.astype(F32))              # [1024, 100]
    xts = np.clip(xT * SA, -FP8_MAX, FP8_MAX)
    xt_arr = np.ascontiguousarray(
        xts.reshape(KD, P, B).transpose(1, 0, 2)).astype(np8)  # [128,8,100]

    W0e, W1e, W2e, W3e, b3e = [], [], [], [], []
    for s in range(STEPS):
        w0 = W0[s] * M0
        if s % 2 == 1:
            w0 = w0[::-1, :]
        w3 = W3[s] * M3
        b3s = b3[s]
        if s % 2 == 1:
            w3 = w3[:, ::-1]
            b3s = b3s[::-1]
        W0e.append(np.ascontiguousarray(w0))
        W1e.append(W1[s] * M1)
        W2e.append(W2[s] * M1)
        W3e.append(np.ascontiguousarray(w3))
        b3e.append(np.ascontiguousarray(b3s))

    l1slots = _l1_slots(0)
    l3slots = _l3_slots()

    in_maps = []
    for c in range(NC):
        ca, cb = c, 15 - c                      # col-tile indices
        sa_ = slice(P * ca, P * (ca + 1))
        sb_ = slice(P * cb, P * (cb + 1))

        # W0: [S, P, 6, 2, P]
        w0c = np.zeros((STEPS, P, sum(SL0), 2, P), dtype=np8)
        for s in range(STEPS):
            slots = _l0_slots(s % 2)
            for i, (k0, k1) in enumerate(slots):
                cols = sa_ if i < SL0[0] else sb_
                w0c[s, :, i, 0, :] = _q8(W0e[s][P * k0:P * (k0 + 1), cols], np8)
                w0c[s, :, i, 1, :] = _q8(W0e[s][P * k1:P * (k1 + 1), cols], np8)

        def pack_h(We):
            wc = np.zeros((STEPS, P, sum(SL1), 2, P), dtype=np8)
            for s in range(STEPS):
                for i, (k0, k1) in enumerate(l1slots):
                    cols = sa_ if i < SL1[0] else sb_
                    wc[s, :, i, 0, :] = _q8(We[s][P * k0:P * (k0 + 1), cols], np8)
                    wc[s, :, i, 1, :] = _q8(We[s][P * k1:P * (k1 + 1), cols], np8)
            return wc

        w1c = pack_h(W1e)
        w2c = pack_h(W2e)

        # W3: [S, P, 8, 2, P]; m-tile = z-block c
        zc = slice(P * c, P * (c + 1))
        w3c = np.zeros((STEPS, P, SL3, 2, P), dtype=np8)
        for s in range(STEPS):
            for i, (k0, k1) in enumerate(l3slots):
                w3c[s, :, i, 0, :] = _q8(W3e[s][P * k0:P * (k0 + 1), zc], np8)
                w3c[s, :, i, 1, :] = _q8(W3e[s][P * k1:P * (k1 + 1), zc], np8)

        # biases: [S, P, 7] f32, cols = b0A,b0B,b1A,b1B,b2A,b2B,b3 (all *SA)
        bc = np.zeros((STEPS, P, 7), dtype=F32)
        for s in range(STEPS):
            bc[s, :, 0] = b0[s][sa_] * SA
            bc[s, :, 1] = b0[s][sb_] * SA
            bc[s, :, 2] = b1[s][sa_] * SA
            bc[s, :, 3] = b1[s][sb_] * SA
            bc[s, :, 4] = b2[s][sa_] * SA
            bc[s, :, 5] = b2[s][sb_] * SA
            bc[s, :, 6] = b3e[s][zc] * SA

        wall = np.concatenate([w0c, w1c, w2c, w3c], axis=2)  # [S,P,38,2,P]
        in_maps.append({
            "xt": np.ascontiguousarray(xt_arr),
            "xloc": np.ascontiguousarray(xT[zc, :] * SA),
            "w": np.ascontiguousarray(wall),
            "bias": np.ascontiguousarray(bc),
        })
    return in_maps




# ---------------- DP8 (pure data-parallel) variant ----------------
BL = 16           # padded per-core batch (8 * 16 = 128 >= 100)


# ---- dp8v2: tight mixed pair/single slot packing ----
def _v2_layer_slots(n_ktiles_of_v, ncols, from_top=False, nk=None):
    """Per col-tile v: list of ('p', k0) DoubleRow pairs / ('s', k) singles."""
    out = []
    for v in range(ncols):
        n = n_ktiles_of_v(v)
        slots = []
        if not from_top:
            k = 0
            while n - k >= 2:
                slots.append(("p", k))
                k += 2
            if k < n:
                slots.append(("s", k))
        else:
            hi = nk
            lo = nk - n
            k = hi
            while k - lo >= 2:
                slots.append(("p", k - 2))
                k -= 2
            if k > lo:
                slots.append(("s", lo))
        out.append((v, slots))
    return out


def _dp8v2_slots(parity):
    """[(layer, v, slots)] for one step. layer in 0..3."""
    table = []
    # L0: col v needs n = v//2+1 ktiles; parity 1 flip-folded -> top tiles
    l0 = _v2_layer_slots(lambda v: v // 2 + 1, 16, from_top=(parity == 1), nk=8)
    for v, sl in l0:
        table.append((0, v, sl))
    # L1/L2: col v needs ktiles 0..v
    l1 = _v2_layer_slots(lambda v: v + 1, 16)
    for ly in (1, 2):
        for v, sl in l1:
            table.append((ly, v, sl))
    # L3: parity0 col v needs ktiles 0..2v+1; parity1 col v = orig 7-v
    def l3n(v):
        vv = v if parity == 0 else v  # cols already flip-folded host-side
        return 2 * vv + 2 if parity == 0 else 2 * (7 - vv) + 2
    l3 = _v2_layer_slots(l3n, 8)
    for v, sl in l3:
        table.append((3, v, sl))
    return table


def _dp8v2_counts(parity):
    np_, ns_ = 0, 0
    for _, _, sl in _dp8v2_slots(parity):
        for kind, _ in sl:
            if kind == "p":
                np_ += 1
            else:
                ns_ += 1
    return np_, ns_


V2_NP, V2_NS = _dp8v2_counts(0)     # pairs, singles per step (parity-invt)
assert (V2_NP, V2_NS) == _dp8v2_counts(1), (_dp8v2_counts(0), _dp8v2_counts(1))


def _prep_inputs_dp8v2(x, W0, b0, W1, b1, W2, b2, W3, b3):
    from concourse import mybir
    np8 = mybir.dt.np(mybir.dt.float8e4)

    M0 = _made_mask(DIM, H, True)
    M1 = _made_mask(H, H, False)
    M3 = _made_mask(H, DIM, False)

    W0e, W1e, W2e, W3e, b3e = [], [], [], [], []
    for s in range(STEPS):
        w0 = W0[s] * M0
        if s % 2 == 1:
            w0 = w0[::-1, :]
        w3 = W3[s] * M3
        b3s = b3[s]
        if s % 2 == 1:
            w3 = w3[:, ::-1]
            b3s = b3s[::-1]
        W0e.append(np.ascontiguousarray(w0))
        W1e.append(W1[s] * M1)
        W2e.append(W2[s] * M1)
        W3e.append(np.ascontiguousarray(w3))
        b3e.append(np.ascontiguousarray(b3s))

    wp = np.zeros((STEPS, P, V2_NP, 2, P), dtype=np8)
    ws = np.zeros((STEPS, P, V2_NS, P), dtype=np8)
    bc = np.zeros((STEPS, P, 56), dtype=F32)
    for s in range(STEPS):
        Wl = [W0e[s], W1e[s], W2e[s], W3e[s]]
        ip = 0
        isg = 0
        for ly, v, sl in _dp8v2_slots(s % 2):
            We = Wl[ly]
            cols = slice(P * v, P * (v + 1))
            for kind, k0 in sl:
                if kind == "p":
                    wp[s, :, ip, 0, :] = _q8(We[P * k0:P * (k0 + 1), cols], np8)
                    wp[s, :, ip, 1, :] = _q8(We[P * (k0 + 1):P * (k0 + 2), cols],
                                             np8)
                    ip += 1
                else:
                    ws[s, :, isg, :] = _q8(We[P * k0:P * (k0 + 1), cols], np8)
                    isg += 1
        assert ip == V2_NP and isg == V2_NS, (ip, isg)
        for v in range(16):
            bc[s, :, v] = b0[s][P * v:P * (v + 1)] * SA
            bc[s, :, 16 + v] = b1[s][P * v:P * (v + 1)] * SA
            bc[s, :, 32 + v] = b2[s][P * v:P * (v + 1)] * SA
        for v in range(8):
            bc[s, :, 48 + v] = b3e[s][P * v:P * (v + 1)] * SA
    wp = np.ascontiguousarray(wp)
    ws = np.ascontiguousarray(ws)
    bc = np.ascontiguousarray(bc)

    xp = np.zeros((NC * BL, DIM), dtype=F32)
    xp[:B] = x * SA
    in_maps = []
    for c in range(NC):
        xc = xp[BL * c:BL * (c + 1)].T          # [DIM, BL]
        zt = xc.reshape(KD, P, BL).transpose(1, 0, 2)   # [P, KD, BL]
        in_maps.append({
            "xloc": np.ascontiguousarray(zt),
            "xt": np.ascontiguousarray(
                np.clip(zt, -FP8_MAX, FP8_MAX).astype(np8)),
            "wp": wp,
            "ws": ws,
            "bias": bc,
        })
    return in_maps


def _build_module_dp8v2(repeat=1):
    from concourse import bass, bacc, tile, mybir

    f32 = mybir.dt.float32
    fp8 = mybir.dt.float8e4
    Relu = mybir.ActivationFunctionType.Relu
    Square = mybir.ActivationFunctionType.Square
    DR = mybir.MatmulPerfMode.DoubleRow

    nc = bacc.Bacc("TRN2", target_bir_lowering=False, debug=False,
                   num_devices=NC)

    xt_d = nc.dram_tensor("xt", [P, KD, BL], fp8, kind="ExternalInput")
    xloc_d = nc.dram_tensor("xloc", [P, KD, BL], f32, kind="ExternalInput")
    wp_d = nc.dram_tensor("wp", [STEPS, P, V2_NP, 2, P], fp8,
                          kind="ExternalInput")
    ws_d = nc.dram_tensor("ws", [STEPS, P, V2_NS, P], fp8,
                          kind="ExternalInput")
    b_d = nc.dram_tensor("bias", [STEPS, P, 56], f32, kind="ExternalInput")
    sq_d = nc.dram_tensor("sq", [1, KD * BL], f32, kind="ExternalOutput")

    # weight-DMA queue layout: "split" = pairs across sync+gpsimd, singles
    # on scalar; "one" = everything on sync (HWDGE)
    WQ = os.environ.get("MAF_WQ", "split")
    RSPLIT = bool(int(os.environ.get("MAF_RSPLIT", "1")))
    # balanced thirds in P-col units: pairs cost 2 cols, singles 1
    if WQ == "split3":
        third = (2 * V2_NP + V2_NS) // 3
        NPA = (third + 1) // 2               # sync gets pairs [0, NPA)
        NPB = NPA + (third + 1) // 2         # gpsimd gets [NPA, NPB)
    elif WQ == "split":
        NPA = (V2_NP + 1) // 2
        NPB = V2_NP
    else:                                    # "one"
        NPA = V2_NP
        NPB = V2_NP

    trace_sim = bool(int(os.environ.get("MAF_TRACE_SIM", "0")))
    with tile.TileContext(nc, trace_sim=trace_sim) as tc:
        with (
            tc.tile_pool(name="w01", bufs=2) as wpool,
            tc.tile_pool(name="hf", bufs=2) as hpool,
            tc.tile_pool(name="zp", bufs=2) as zpool,
            tc.tile_pool(name="loc", bufs=2) as locpool,
            tc.tile_pool(name="cst", bufs=1) as cpool,
            tc.tile_pool(name="ps", bufs=1, space=bass.MemorySpace.PSUM) as pspool,
        ):
            ones = cpool.tile([P, 1], f32, tag="ones")
            nc.gpsimd.memset(ones[:], 1.0)
            bias_t = cpool.tile([P, STEPS, 56], f32, tag="bias")
            nc.sync.dma_start(bias_t[:], b_d.rearrange("s p k -> p s k"))

            zT = zpool.tile([P, KD, BL], fp8, tag="zT")
            nc.sync.dma_start(zT[:], xt_d[:])
            zloc = zpool.tile([P, KD, BL], f32, tag="zloc")
            nc.sync.dma_start(zloc[:], xloc_d[:])

            for it in range(STEPS * repeat):
                s = it % STEPS
                wA = wpool.tile([P, NPA, 2, P], fp8, tag="wA")
                nc.sync.dma_start(wA[:], wp_d[s, :, :NPA])
                wB = wC = None
                if NPB > NPA:
                    wB = wpool.tile([P, NPB - NPA, 2, P], fp8, tag="wB")
                    nc.gpsimd.dma_start(wB[:], wp_d[s, :, NPA:NPB])
                if V2_NP > NPB:
                    wC = wpool.tile([P, V2_NP - NPB, 2, P], fp8, tag="wC")
                    nc.scalar.dma_start(wC[:], wp_d[s, :, NPB:])
                wS = wpool.tile([P, V2_NS, P], fp8, tag="wS")
                (nc.scalar if WQ != "one" else nc.sync).dma_start(
                    wS[:], ws_d[s])

                def wpair(i):
                    if i < NPA:
                        return wA[:, i, :, :]
                    if i < NPB:
                        return wB[:, i - NPA, :, :]
                    return wC[:, i - NPB, :, :]

                src_of = [zT, None, None, None]
                bcol_of = [0, 16, 32, 48]
                ip = 0
                isg = 0
                h_out = None
                shf = locpool.tile([P, KD, BL], f32, tag="shf")
                cur_ly = -1
                for ly, v, sl in _dp8v2_slots(s % 2):
                    if ly != cur_ly:
                        cur_ly = ly
                        if ly in (1, 2, 3):
                            src = h_out
                        else:
                            src = zT
                        if ly < 3:
                            h_out = hpool.tile([P, 16, BL], fp8, tag=f"h{ly}T")
                    pb = pspool.tile([P, BL], f32, tag=f"pb{v % 8}")
                    nsl = len(sl)
                    for j, (kind, k0) in enumerate(sl):
                        st = (j == 0)
                        sp = (j == nsl - 1)
                        if kind == "p":
                            nc.tensor.matmul(pb[:], wpair(ip),
                                             src[:, k0:k0 + 2, :],
                                             start=st, stop=sp, perf_mode=DR)
                            ip += 1
                        else:
                            nc.tensor.matmul(pb[:], wS[:, isg, :],
                                             src[:, k0, :],
                                             start=st, stop=sp)
                            isg += 1
                    if ly < 3:
                        bap = bias_t[:, s, bcol_of[ly] + v:bcol_of[ly] + v + 1]
                        if RSPLIT and v % 2 == 1:
                            # DVE path: (pb/SW + b) then max(.,0), fp8 out
                            tmp = locpool.tile([P, BL], f32, tag="rtmp")
                            nc.vector.tensor_scalar(tmp[:], pb[:], 1.0 / SW,
                                                    bap,
                                                    mybir.AluOpType.mult,
                                                    mybir.AluOpType.add)
                            nc.vector.tensor_scalar(h_out[:, v, :], tmp[:],
                                                    0.0, None,
                                                    mybir.AluOpType.max)
                        else:
                            nc.scalar.activation(h_out[:, v, :], pb[:], Relu,
                                                 bias=bap, scale=1.0 / SW)
                    else:
                        nc.vector.tensor_scalar(shf[:, v, :], pb[:], 1.0 / SW,
                                                bias_t[:, s, 48 + v:48 + v + 1],
                                                mybir.AluOpType.mult,
                                                mybir.AluOpType.add)
                zloc2 = zpool.tile([P, KD, BL], f32, tag="zloc")
                nc.vector.tensor_sub(zloc2[:], zloc[:], shf[:])
                if it != STEPS * repeat - 1:
                    zT = zpool.tile([P, KD, BL], fp8, tag="zT")
                    nc.vector.tensor_sub(zT[:], zloc[:], shf[:])
                zloc = zloc2

            z2 = locpool.tile([P, KD, BL], f32, tag="z2")
            nc.scalar.activation(z2[:], zloc[:], Square, scale=1.0 / SA)
            psq = pspool.tile([1, KD * BL], f32, tag="pb0")
            nc.tensor.matmul(psq[:], ones[:], z2[:].opt(), start=True,
                             stop=True)
            sq_sb = locpool.tile([1, KD * BL], f32, tag="sqsb")
            nc.vector.tensor_copy(sq_sb[:], psq[:])
            nc.sync.dma_start(sq_d[:], sq_sb[:])

    nc.compile()
    return nc

def _dp8_l0_slots(parity):
    """[(m_tile, [(k0, k1), ...])] for the full W0 of one step."""
    out = []
    for v in range(16):
        q = (v // 2 + 1 + 1) // 2 if False else -(-(v // 2 + 1) // 2)
        if parity == 0:
            prs = list(range(q))
        else:
            prs = [3 - t for t in range(q)]
        out.append((v, [(2 * p, 2 * p + 1) for p in prs]))
    return out


def _dp8_l1_slots():
    out = []
    for v in range(16):
        q = -(-(v + 1) // 2)
        out.append((v, [(2 * t, 2 * t + 1) for t in range(q)]))
    return out


def _dp8_l3_slots(parity):
    out = []
    for v in range(8):
        n = (v + 1) if parity == 0 else (8 - v)
        out.append((v, [(2 * t, 2 * t + 1) for t in range(n)]))
    return out


DP8_NS_L0 = sum(len(p) for _, p in _dp8_l0_slots(0))   # 40
DP8_NS_L1 = sum(len(p) for _, p in _dp8_l1_slots())    # 72
DP8_NS_L3 = sum(len(p) for _, p in _dp8_l3_slots(0))   # 36
DP8_NS = DP8_NS_L0 + 2 * DP8_NS_L1 + DP8_NS_L3         # 220


def _prep_inputs_dp8(x, W0, b0, W1, b1, W2, b2, W3, b3):
    from concourse import mybir
    np8 = mybir.dt.np(mybir.dt.float8e4)

    M0 = _made_mask(DIM, H, True)
    M1 = _made_mask(H, H, False)
    M3 = _made_mask(H, DIM, False)

    W0e, W1e, W2e, W3e, b3e = [], [], [], [], []
    for s in range(STEPS):
        w0 = W0[s] * M0
        if s % 2 == 1:
            w0 = w0[::-1, :]
        w3 = W3[s] * M3
        b3s = b3[s]
        if s % 2 == 1:
            w3 = w3[:, ::-1]
            b3s = b3s[::-1]
        W0e.append(np.ascontiguousarray(w0))
        W1e.append(W1[s] * M1)
        W2e.append(W2[s] * M1)
        W3e.append(np.ascontiguousarray(w3))
        b3e.append(np.ascontiguousarray(b3s))

    wall = np.zeros((STEPS, P, DP8_NS, 2, P), dtype=np8)
    bc = np.zeros((STEPS, P, 56), dtype=F32)
    for s in range(STEPS):
        i = 0
        for v, prs in _dp8_l0_slots(s % 2):
            cols = slice(P * v, P * (v + 1))
            for (k0, k1) in prs:
                wall[s, :, i, 0, :] = _q8(W0e[s][P * k0:P * (k0 + 1), cols], np8)
                wall[s, :, i, 1, :] = _q8(W0e[s][P * k1:P * (k1 + 1), cols], np8)
                i += 1
        for We in (W1e, W2e):
            for v, prs in _dp8_l1_slots():
                cols = slice(P * v, P * (v + 1))
                for (k0, k1) in prs:
                    wall[s, :, i, 0, :] = _q8(We[s][P * k0:P * (k0 + 1), cols], np8)
                    wall[s, :, i, 1, :] = _q8(We[s][P * k1:P * (k1 + 1), cols], np8)
                    i += 1
        for v, prs in _dp8_l3_slots(s % 2):
            cols = slice(P * v, P * (v + 1))
            for (k0, k1) in prs:
                wall[s, :, i, 0, :] = _q8(W3e[s][P * k0:P * (k0 + 1), cols], np8)
                wall[s, :, i, 1, :] = _q8(W3e[s][P * k1:P * (k1 + 1), cols], np8)
                i += 1
        assert i == DP8_NS
        for v in range(16):
            bc[s, :, v] = b0[s][P * v:P * (v + 1)] * SA
            bc[s, :, 16 + v] = b1[s][P * v:P * (v + 1)] * SA
            bc[s, :, 32 + v] = b2[s][P * v:P * (v + 1)] * SA
        for v in range(8):
            bc[s, :, 48 + v] = b3e[s][P * v:P * (v + 1)] * SA
    wall = np.ascontiguousarray(wall)
    bc = np.ascontiguousarray(bc)

    xp = np.zeros((NC * BL, DIM), dtype=F32)
    xp[:B] = x * SA
    in_maps = []
    for c in range(NC):
        xc = xp[BL * c:BL * (c + 1)].T          # [DIM, BL]
        zt = xc.reshape(KD, P, BL).transpose(1, 0, 2)   # [P, KD, BL]
        in_maps.append({
            "xloc": np.ascontiguousarray(zt),
            "xt": np.ascontiguousarray(
                np.clip(zt, -FP8_MAX, FP8_MAX).astype(np8)),
            "w": wall,
            "bias": bc,
        })
    return in_maps


def _build_module_dp8(repeat=1):
    from concourse import bass, bacc, tile, mybir

    f32 = mybir.dt.float32
    fp8 = mybir.dt.float8e4
    Relu = mybir.ActivationFunctionType.Relu
    Square = mybir.ActivationFunctionType.Square
    DR = mybir.MatmulPerfMode.DoubleRow

    nc = bacc.Bacc("TRN2", target_bir_lowering=False, debug=False,
                   num_devices=NC)

    NA = DP8_NS_L0 + DP8_NS_L1               # chunk A: L0 + L1 (112)
    xt_d = nc.dram_tensor("xt", [P, KD, BL], fp8, kind="ExternalInput")
    xloc_d = nc.dram_tensor("xloc", [P, KD, BL], f32, kind="ExternalInput")
    w_d = nc.dram_tensor("w", [STEPS, P, DP8_NS, 2, P], fp8,
                         kind="ExternalInput")
    b_d = nc.dram_tensor("bias", [STEPS, P, 56], f32, kind="ExternalInput")
    sq_d = nc.dram_tensor("sq", [1, KD * BL], f32, kind="ExternalOutput")

    trace_sim = bool(int(os.environ.get("MAF_TRACE_SIM", "0")))
    with tile.TileContext(nc, trace_sim=trace_sim) as tc:
        with (
            tc.tile_pool(name="w01", bufs=2) as wpool,
            tc.tile_pool(name="hf", bufs=2) as hpool,
            tc.tile_pool(name="zp", bufs=2) as zpool,
            tc.tile_pool(name="loc", bufs=2) as locpool,
            tc.tile_pool(name="cst", bufs=1) as cpool,
            tc.tile_pool(name="ps", bufs=1, space=bass.MemorySpace.PSUM) as pspool,
        ):
            ones = cpool.tile([P, 1], f32, tag="ones")
            nc.gpsimd.memset(ones[:], 1.0)
            bias_t = cpool.tile([P, STEPS, 56], f32, tag="bias")
            nc.sync.dma_start(bias_t[:], b_d.rearrange("s p k -> p s k"))

            zT = zpool.tile([P, KD, BL], fp8, tag="zT")
            nc.sync.dma_start(zT[:], xt_d[:])
            zloc = zpool.tile([P, KD, BL], f32, tag="zloc")
            nc.sync.dma_start(zloc[:], xloc_d[:])

            l1slots = _dp8_l1_slots()

            for it in range(STEPS * repeat):
                s = it % STEPS
                wA = wpool.tile([P, NA, 2, P], fp8, tag="wA")
                nc.sync.dma_start(wA[:], w_d[s, :, :NA])
                wB = wpool.tile([P, DP8_NS - NA, 2, P], fp8, tag="wB")
                nc.gpsimd.dma_start(wB[:], w_d[s, :, NA:])

                def zpair(k0):
                    return zT[:, k0:k0 + 2, :]

                # L0
                h0T = hpool.tile([P, 16, BL], fp8, tag="h0T")
                i = 0
                for v, prs in _dp8_l0_slots(s % 2):
                    pb = pspool.tile([P, BL], f32, tag=f"pb{v % 8}")
                    for j, (k0, k1) in enumerate(prs):
                        nc.tensor.matmul(pb[:], wA[:, i, :, :], zpair(k0),
                                         start=(j == 0),
                                         stop=(j == len(prs) - 1),
                                         perf_mode=DR)
                        i += 1
                    nc.scalar.activation(h0T[:, v, :], pb[:], Relu,
                                         bias=bias_t[:, s, v:v + 1],
                                         scale=1.0 / SW)

                def h_layer(src, wtile, base, bcol, tag):
                    out = hpool.tile([P, 16, BL], fp8, tag=tag)
                    i = base
                    for v, prs in l1slots:
                        pb = pspool.tile([P, BL], f32, tag=f"pb{v % 8}")
                        for j, (k0, k1) in enumerate(prs):
                            nc.tensor.matmul(pb[:], wtile[:, i, :, :],
                                             src[:, k0:k0 + 2, :],
                                             start=(j == 0),
                                             stop=(j == len(prs) - 1),
                                             perf_mode=DR)
                            i += 1
                        nc.scalar.activation(out[:, v, :], pb[:], Relu,
                                             bias=bias_t[:, s,
                                                         bcol + v:bcol + v + 1],
                                             scale=1.0 / SW)
                    return out

                h1T = h_layer(h0T, wA, DP8_NS_L0, 16, "h1T")
                h2T = h_layer(h1T, wB, 0, 32, "h2T")

                # L3 + z update
                shf = locpool.tile([P, KD, BL], f32, tag="shf")
                i = DP8_NS_L1
                for v, prs in _dp8_l3_slots(s % 2):
                    pb = pspool.tile([P, BL], f32, tag=f"pb{v % 8}")
                    for j, (k0, k1) in enumerate(prs):
                        nc.tensor.matmul(pb[:], wB[:, i, :, :],
                                         h2T[:, k0:k0 + 2, :],
                                         start=(j == 0),
                                         stop=(j == len(prs) - 1),
                                         perf_mode=DR)
                        i += 1
                    nc.vector.tensor_scalar(shf[:, v, :], pb[:], 1.0 / SW,
                                            bias_t[:, s, 48 + v:48 + v + 1],
                                            mybir.AluOpType.mult,
                                            mybir.AluOpType.add)
                zloc2 = zpool.tile([P, KD, BL], f32, tag="zloc")
                nc.vector.tensor_sub(zloc2[:], zloc[:], shf[:])
                if it != STEPS * repeat - 1:
                    zT = zpool.tile([P, KD, BL], fp8, tag="zT")
                    nc.vector.tensor_sub(zT[:], zloc[:], shf[:])
                zloc = zloc2

            z2 = locpool.tile([P, KD, BL], f32, tag="z2")
            nc.scalar.activation(z2[:], zloc[:], Square, scale=1.0 / SA)
            psq = pspool.tile([1, KD * BL], f32, tag="pb0")
            nc.tensor.matmul(psq[:], ones[:], z2[:].opt(), start=True,
                             stop=True)
            sq_sb = locpool.tile([1, KD * BL], f32, tag="sqsb")
            nc.vector.tensor_copy(sq_sb[:], psq[:])
            nc.sync.dma_start(sq_d[:], sq_sb[:])

    nc.compile()
    return nc


_CACHED_NC = {}


IMPL = os.environ.get("MAF_IMPL", "dp8v2")


def _build_module(repeat=1):
    """Build the SPMD module. repeat>1 runs the whole MAF body N times
    back-to-back (timing builds only; output is then meaningless)."""
    key = (IMPL, repeat)
    if key in _CACHED_NC:
        return _CACHED_NC[key]
    if IMPL == "dp8":
        nc = _build_module_dp8(repeat)
        _CACHED_NC[key] = nc
        return nc
    if IMPL == "dp8v2":
        nc = _build_module_dp8v2(repeat)
        _CACHED_NC[key] = nc
        return nc

    from concourse import bass, bacc, tile, mybir

    f32 = mybir.dt.float32
    fp8 = mybir.dt.float8e4
    Relu = mybir.ActivationFunctionType.Relu
    Copy = mybir.ActivationFunctionType.Copy
    Square = mybir.ActivationFunctionType.Square
    DR = mybir.MatmulPerfMode.DoubleRow
    RG = [list(range(NC))]
    no_cc = bool(int(os.environ.get("MAF_NO_CC", "0")))  # timing ablation

    nc = bacc.Bacc("TRN2", target_bir_lowering=False, debug=False,
                   num_devices=NC)

    NSLOT = sum(SL0) + 2 * sum(SL1) + SL3  # 6 + 24 + 8 = 38
    xt_d = nc.dram_tensor("xt", [P, KD, B], fp8, kind="ExternalInput")
    xloc_d = nc.dram_tensor("xloc", [P, B], f32, kind="ExternalInput")
    w_d = nc.dram_tensor("w", [STEPS, P, NSLOT, 2, P], fp8,
                         kind="ExternalInput")
    b_d = nc.dram_tensor("bias", [STEPS, P, 7], f32, kind="ExternalInput")
    sq_d = nc.dram_tensor("sq", [1, B], f32, kind="ExternalOutput")

    trace_sim = bool(int(os.environ.get("MAF_TRACE_SIM", "0")))
    with tile.TileContext(nc, trace_sim=trace_sim) as tc:
        with (
            tc.tile_pool(name="w01", bufs=2) as wpool,
            tc.tile_pool(name="hf", bufs=2) as hpool,
            tc.tile_pool(name="zp", bufs=2) as zpool,
            tc.tile_pool(name="loc", bufs=2) as locpool,
            tc.tile_pool(name="cst", bufs=1) as cpool,
            tc.tile_pool(name="ps", bufs=2, space=bass.MemorySpace.PSUM) as pspool,
            tc.tile_pool(name="drb", bufs=2, space="DRAM") as dpool,
        ):
            ones = cpool.tile([P, 1], f32, tag="ones")
            nc.gpsimd.memset(ones[:], 1.0)
            bias_t = cpool.tile([P, STEPS, 7], f32, tag="bias")
            nc.sync.dma_start(bias_t[:], b_d.rearrange("s p k -> p s k"))

            zT = zpool.tile([P, KD, B], fp8, tag="zT")
            nc.sync.dma_start(zT[:], xt_d[:])
            zloc = zpool.tile([P, B], f32, tag="zloc")
            nc.sync.dma_start(zloc[:], xloc_d[:])

            def allgather_h(hloc, out_tag):
                agi = dpool.tile([P, 2, B], fp8, tag="agi")
                # bounce on the scalar queue: same engine as the producing
                # activations, so no cross-engine semaphore hop
                nc.scalar.dma_start(agi[:], hloc[:])
                ago = dpool.tile([4, 2, P, 2, B], fp8, tag="ago")
                if no_cc:
                    nc.sync.dma_start(ago[0, 0, :, :, :], agi[:])
                else:
                    nc.gpsimd.collective_compute(
                        "AllGather", mybir.AluOpType.bypass, replica_groups=RG,
                        ins=[agi.opt()], outs=[ago.opt()])
                hT = hpool.tile([P, 2, 4, 2, B], fp8, tag=out_tag)
                nc.sync.dma_start(hT[:], ago.rearrange("u j p m b -> p m u j b"))
                return hT

            def mm_group(ps, w_t, s, s0, rhs_list, tag):
                n = len(rhs_list)
                for u in range(n):
                    nc.tensor.matmul(ps[:], w_t[:, s, s0 + u, :, :],
                                     rhs_list[u],
                                     start=(u == 0), stop=(u == n - 1),
                                     perf_mode=DR)

            def h_layer(s, w_t, off, bcol, rhsA, rhsB, out_tag):
                hloc = locpool.tile([P, 2, B], fp8, tag="hloc")
                psA = pspool.tile([P, B], f32, tag="psA")
                mm_group(psA, w_t, s, off, rhsA, "A")
                nc.scalar.activation(hloc[:, 0, :], psA[:], Relu,
                                     bias=bias_t[:, s, bcol:bcol + 1],
                                     scale=1.0 / SW)
                psB = pspool.tile([P, B], f32, tag="psB")
                mm_group(psB, w_t, s, off + len(rhsA), rhsB, "B")
                nc.scalar.activation(hloc[:, 1, :], psB[:], Relu,
                                     bias=bias_t[:, s, bcol + 1:bcol + 2],
                                     scale=1.0 / SW)
                return allgather_h(hloc, out_tag)

            wall = None
            for it in range(STEPS * repeat):
                s = it % STEPS
                is_last = it == STEPS * repeat - 1
                if s == 0:
                    # one burst DMA per body: all 4 steps' weights (39 KB
                    # per partition) -> no per-step weight traffic contending
                    # with the AG chain
                    wall = wpool.tile([P, STEPS, NSLOT, 2, P], fp8, tag="w")
                    nc.scalar.dma_start(
                        wall[:], w_d.rearrange("s p n j q -> p s n j q"))
                o1 = sum(SL0)
                o2 = o1 + sum(SL1)
                o3 = o2 + sum(SL1)

                def zpair(t):
                    return zT[:, 2 * t:2 * t + 2, :]

                pa = (0, 1) if s % 2 == 0 else (3, 2)
                h0T = h_layer(s, wall, 0, 0,
                              [zpair(pa[0]), zpair(pa[1])],
                              [zpair(t) for t in range(4)], "h0T")

                def hpair(hT, hg, u):
                    return hT[:, hg, u, :, :]

                rhsA1 = [hpair(h0T, 0, u) for u in range(4)]
                rhsB1 = rhsA1 + [hpair(h0T, 1, u) for u in range(4)]
                h1T = h_layer(s, wall, o1, 2, rhsA1, rhsB1, "h1T")

                rhsA2 = [hpair(h1T, 0, u) for u in range(4)]
                rhsB2 = rhsA2 + [hpair(h1T, 1, u) for u in range(4)]
                h2T = h_layer(s, wall, o2, 4, rhsA2, rhsB2, "h2T")

                rhs3 = [hpair(h2T, 0, u) for u in range(4)] + \
                       [hpair(h2T, 1, u) for u in range(4)]
                ps3 = pspool.tile([P, B], f32, tag="psA")
                mm_group(ps3, wall, s, o3, rhs3, "L3")
                shf = locpool.tile([P, B], f32, tag="shf")
                # shf = ps3/SW + b3*SA (scalars are per-partition APs)
                nc.vector.tensor_scalar(shf[:], ps3[:], 1.0 / SW,
                                        bias_t[:, s, 6:7],
                                        mybir.AluOpType.mult,
                                        mybir.AluOpType.add)
                zloc2 = zpool.tile([P, B], f32, tag="zloc")
                nc.vector.tensor_sub(zloc2[:], zloc[:], shf[:])

                if not is_last:
                    # fp8 copy for the AG computed directly (not serially
                    # after zloc2) so the z critical path has one vector op
                    zlb = locpool.tile([P, B], fp8, tag="zlb")
                    nc.vector.tensor_sub(zlb[:], zloc[:], shf[:])
                    zin = dpool.tile([P, B], fp8, tag="zin")
                    nc.sync.dma_start(zin[:], zlb[:])
                    zout = dpool.tile([NC, P, B], fp8, tag="zout")
                    if no_cc:
                        nc.sync.dma_start(zout[0, :, :], zin[:])
                    else:
                        nc.gpsimd.collective_compute(
                            "AllGather", mybir.AluOpType.bypass,
                            replica_groups=RG,
                            ins=[zin.opt()], outs=[zout.opt()])
                    zT = zpool.tile([P, KD, B], fp8, tag="zT")
                    nc.sync.dma_start(zT[:], zout.rearrange("c p b -> p c b"))
                zloc = zloc2

            z2 = locpool.tile([P, B], f32, tag="z2")
            nc.scalar.activation(z2[:], zloc[:], Square, scale=1.0 / SA)
            psq = pspool.tile([1, B], f32, tag="psq")
            nc.tensor.matmul(psq[:], ones[:], z2[:], start=True, stop=True)
            sq_sb = locpool.tile([1, B], f32, tag="sqsb")
            nc.vector.tensor_copy(sq_sb[:], psq[:])
            nc.sync.dma_start(sq_d[:], sq_sb[:])

    nc.compile()
    _CACHED_NC[(IMPL, repeat)] = nc
    return nc


def kernel(x, W0, b0, W1, b1, W2, b2, W3, b3):
    from concourse import bass_utils

    if IMPL == "dp8":
        in_maps = _prep_inputs_dp8(x, W0, b0, W1, b1, W2, b2, W3, b3)
    elif IMPL == "dp8v2":
        in_maps = _prep_inputs_dp8v2(x, W0, b0, W1, b1, W2, b2, W3, b3)
    else:
        in_maps = _prep_inputs(x, W0, b0, W1, b1, W2, b2, W3, b3)
    nc = _build_module()
    res = bass_utils.run_bass_kernel_spmd(
        nc, in_maps, core_ids=list(range(NC)),
        trace=bool(int(os.environ.get("MAF_TRACE", "0"))))
    if IMPL in ("dp8", "dp8v2"):
        sq = np.concatenate([
            res.results[c]["sq"].reshape(KD, BL).astype(np.float64).sum(0)
            for c in range(NC)])[:B]
        out = 0.5 * sq + 0.5 * DIM * LOG_2PI
    else:
        total = np.zeros(B, dtype=np.float64)
        for c in range(NC):
            total += res.results[c]["sq"][0].astype(np.float64)
        out = 0.5 * total + 0.5 * DIM * LOG_2PI
    if res.exec_time_ns is not None:
        kernel.last_exec_time_ns = res.exec_time_ns
    return out.astype(F32)


kernel.last_exec_time_ns = None

